# revision 12
# baseline (speedup 1.0000x reference)
"""Trainium2 Bass kernel for nn_MoDEEncoderBlock (MoDE encoder block).

Sharding: 8 cores = 2 samples x 4 D-slabs (16 output planes each).
Each core runs: mode_conv1 (with 2-plane halo recompute) -> instance_norm+mish
-> mode_conv2 -> instance_norm+mish (= x_skip) -> strided down conv ->
instance_norm+mish (= y). Instance-norm statistics are globalized with a tiny
AllReduce among each sample's 4 cores.

Conv = 125 shifted matmuls accumulating in PSUM, K-stacked (layer1: 4 kh-shifted
copies x 32ci = K128 quads; layer2: 2 copies x 64ci = K128 pairs), with two
512-position output blocks running concurrently via PE column tiling. Matmuls
use float32r (full-rate fp32).
"""
import os
import sys

for _p in ("/opt/trn_rl_repo", "/root/.axon_site/_ro/trn_rl_repo"):
    if os.path.isdir(_p) and _p not in sys.path:
        sys.path.insert(0, _p)

import numpy as np
import concourse.bacc as bacc
import concourse.mybir as mybir
import concourse.tile as tile
from concourse import bass_utils

f32 = mybir.dt.float32
f32r = mybir.dt.float32r
AF = mybir.ActivationFunctionType
ALU = mybir.AluOpType

EPS = 1e-5
CO = 64
CI1 = 32
T_DIM = 10
RG = [[0, 1, 2, 3], [4, 5, 6, 7]]

FULL = dict(D=64, H=64, W=64)
MINI = dict(D=16, H=16, W=16)


# ----------------------------------------------------------------------------
# host-side weight preparation (pure layout/static-weight transforms)
# ----------------------------------------------------------------------------

def _pad_k(k, p):
    return np.pad(k, ((0, 0), (0, 0), (p, p), (p, p), (p, p)))


def expert_stack(c5, c3, c1, a3, a5):
    """(5, Co, Ci, 5, 5, 5) expert kernel stack, mirrors reference."""
    pool3 = np.full((3, 3, 3), np.float32(1.0) / np.float32(27.0), np.float32)
    pool5 = np.full((5, 5, 5), np.float32(1.0) / np.float32(125.0), np.float32)
    return np.stack([
        c5,
        _pad_k(c3, 1),
        _pad_k(c1, 2),
        _pad_k(a3 * pool3, 1),
        a5 * pool5,
    ], axis=0).astype(np.float32)


def expert_lhsT_l1(E):
    """(5, 128, 34*64): kh0-3 quads (partition-stacked) + kh=4 singles
    packed 3-per-column on row-groups 0/32/64."""
    quad = E[:, :, :, :, 0:4, :]                       # e co ci kd j kw
    quad = quad.transpose(0, 4, 2, 3, 5, 1)            # e j ci kd kw co
    quad = quad.reshape(5, 4 * CI1, 25 * CO)
    sing = E[:, :, :, :, 4, :]                         # e co ci kd kw
    sing = sing.transpose(0, 2, 3, 4, 1).reshape(5, CI1, 25, CO)
    sing_f = np.zeros((5, 128, 9 * CO), np.float32)
    for i, (g, sl) in enumerate(((0, slice(0, 9)), (1, slice(9, 17)),
                                 (2, slice(17, 25)))):
        cnt = sl.stop - sl.start
        sing_f[:, 32 * g:32 * g + CI1, 0:cnt * CO] = \
            sing[:, :, sl].reshape(5, CI1, cnt * CO)
    return np.concatenate([quad, sing_f], axis=2).astype(np.float32)


def expert_lhsT_l2(E):
    """(5, 128, 63*64): kh-pairs (2-stacked) + kh=4 singles packed
    2-per-column on row-groups 0/64 (13 + 12)."""
    ci = CO
    pair = E[:, :, :, :, 0:4, :].reshape(5, CO, ci, 5, 2, 2, 5)  # e co ci kd half j kw
    pair = pair.transpose(0, 5, 2, 3, 6, 4, 1)                   # e j ci kd kw half co
    pair = pair.reshape(5, 2 * ci, 50 * CO)
    sing = E[:, :, :, :, 4, :].transpose(0, 2, 3, 4, 1).reshape(5, ci, 25, CO)
    sing_f = np.zeros((5, 128, 13 * CO), np.float32)
    sing_f[:, 0:ci, 0:13 * CO] = sing[:, :, 0:13].reshape(5, ci, 13 * CO)
    sing_f[:, ci:2 * ci, 0:12 * CO] = sing[:, :, 13:25].reshape(5, ci, 12 * CO)
    return np.concatenate([pair, sing_f], axis=2).astype(np.float32)


def down_lhsT(dw):
    """(128, 4*64): kd-pairs stacked; col (kh*2+kw)*64+co."""
    # dw: (Co, Ci=64, 2, 2, 2)
    a = dw.transpose(2, 1, 3, 4, 0)          # kd ci kh kw co
    return a.reshape(128, 4 * CO).astype(np.float32)


def gate_lhsT(gw, gb):
    """(11, 5*64): row k<10 = gw[:, k]; row 10 = gb."""
    out = np.zeros((T_DIM + 1, 5 * CO), np.float32)
    out[:T_DIM] = gw.T
    out[T_DIM] = gb
    return out


# ----------------------------------------------------------------------------
# device program
# ----------------------------------------------------------------------------

def build_program(cfg):
    D, H, W = cfg["D"], cfg["H"], cfg["W"]
    slab = D // 4
    HP, WP = H + 4, W + 4
    PS = HP * WP
    P1 = slab + 4            # conv1 output planes (incl 2+2 halo)
    PX = slab + 8            # x input planes
    nblk = H // 8            # 8-row blocks per plane
    npair = nblk // 2
    NB = 8 * W               # psum free size per block
    HD, WD = H // 2, W // 2
    dbr = min(HD, 512 // WD)     # down block rows
    nblkd = HD // dbr            # blocks per down plane (2 full, 1 mini)
    DP = slab // 2
    NBD = dbr * WD
    N1 = D * H * W
    ND = (D // 2) * (H // 2) * (W // 2)
    L1C = 34 * CO
    L2C = 63 * CO

    nc = bacc.Bacc("TRN2", target_bir_lowering=False, debug=False,
                   enable_asserts=False, num_devices=8)

    # ---- I/O ----
    xin = nc.dram_tensor("xin", [CI1, PX, HP, WP], f32, kind="ExternalInput").ap()
    taug = nc.dram_tensor("taug", [T_DIM + 1, 1], f32, kind="ExternalInput").ap()
    e1 = nc.dram_tensor("e1", [5, 128, L1C], f32, kind="ExternalInput").ap()
    e2 = nc.dram_tensor("e2", [5, 128, L2C], f32, kind="ExternalInput").ap()
    gw1 = nc.dram_tensor("gw1", [T_DIM + 1, 5 * CO], f32, kind="ExternalInput").ap()
    gw2 = nc.dram_tensor("gw2", [T_DIM + 1, 5 * CO], f32, kind="ExternalInput").ap()
    wdn = nc.dram_tensor("wdn", [128, 4 * CO], f32, kind="ExternalInput").ap()
    aff = nc.dram_tensor("aff", [CO, 6], f32, kind="ExternalInput").ap()
    mask = nc.dram_tensor("mask", [CO, 2], f32, kind="ExternalInput").ap()
    xskip = nc.dram_tensor("xskip", [CO, slab, H, W], f32, kind="ExternalOutput").ap()
    yout = nc.dram_tensor("yout", [CO, DP, HD, WD], f32, kind="ExternalOutput").ap()

    from contextlib import ExitStack
    with tile.TileContext(nc) as tc, ExitStack() as ctx:
        plane = ctx.enter_context(tc.tile_pool(name="plane", bufs=5))
        wrp = ctx.enter_context(tc.tile_pool(name="wrp", bufs=1))
        chp = ctx.enter_context(tc.tile_pool(name="chp", bufs=2))
        evp = ctx.enter_context(tc.tile_pool(name="evp", bufs=3))
        stp = ctx.enter_context(tc.tile_pool(name="stp", bufs=1))
        nop = ctx.enter_context(tc.tile_pool(name="nop", bufs=3))
        gp = ctx.enter_context(tc.tile_pool(name="gp", bufs=1))
        ps = ctx.enter_context(tc.tile_pool(name="ps", bufs=2, space="PSUM"))
        psg = ctx.enter_context(tc.tile_pool(name="psg", bufs=1, space="PSUM"))
        dr = ctx.enter_context(tc.tile_pool(name="dr", bufs=1, space="DRAM"))

        # ---- DRAM scratch ----
        y1_d = [dr.tile([CO, H * W], f32, tag=f"y1_{p}", name=f"y1_{p}") for p in range(P1)]
        hext = [dr.tile([CO, PS], f32, tag=f"hx_{p}", name=f"hx_{p}") for p in range(P1)]
        y2_d = [dr.tile([CO, H * W], f32, tag=f"y2_{p}", name=f"y2_{p}") for p in range(slab)]
        y3_d = [dr.tile([CO, nblkd * NBD], f32, tag=f"y3_{p}", name=f"y3_{p}") for p in range(DP)]
        w_d = [dr.tile([CO, H * W], f32, tag=f"wd_{p}", name=f"wd_{p}") for p in range(P1)]
        w3_d = [dr.tile([CO, nblkd * NBD], f32, tag=f"w3_{p}", name=f"w3_{p}") for p in range(DP)]
        g_d = [dr.tile([CO, 8], f32, tag=f"g_{l}", name=f"g_{l}") for l in range(2)]
        wsyn_d = [dr.tile([128, L1C], f32, tag="ws1", name="ws1"),
                  dr.tile([128, L2C], f32, tag="ws2", name="ws2")]
        ar_in = [dr.tile([CO, 2], f32, tag=f"ari{l}", name=f"ari{l}") for l in range(3)]
        ar_out = [dr.tile([CO, 2], f32, tag=f"aro{l}", name=f"aro{l}")
                  for l in range(3)]

        # ---- constants / small tiles ----
        afft = gp.tile([CO, 6], f32, tag="afft")
        nc.sync.dma_start(afft[:], aff)
        maskt = gp.tile([CO, 2], f32, tag="maskt")
        nc.sync.dma_start(maskt[:], mask)
        taut = gp.tile([T_DIM + 1, 1], f32, tag="taut")
        nc.sync.dma_start(taut[:], taug)
        ones1 = gp.tile([1, 128], f32, tag="ones1")
        nc.vector.memset(ones1[:], 1.0)

        # zero h_ext padding (borders). Use a zero tile, DMA over all hext planes.
        zt = nop.tile([CO, 2048], f32, tag="ni")
        nc.vector.memset(zt[:], 0.0)
        for p in range(P1):
            c0 = 0
            while c0 < PS:
                cw = min(2048, PS - c0)
                nc.sync.dma_start(hext[p][:, c0:c0 + cw], zt[:, 0:cw])
                c0 += cw

        # ---------------- gate + weight synthesis (per mode_conv layer) -------
        def gate_and_synth(l, gw_ap, e_ap, COLS, wr_tile):
            gwt = gp.tile([T_DIM + 1, 5 * CO], f32, tag="gwt")
            nc.sync.dma_start(gwt[:], gw_ap)
            zg = psg.tile([CO, 512], f32, tag="zg")
            for e in range(5):
                nc.tensor.matmul(zg[:, e:e + 1], gwt[0:T_DIM + 1, e * CO:(e + 1) * CO],
                                 taut[0:T_DIM + 1, 0:1],
                                 start=(e == 0), stop=(e == 4), skip_group_check=True)
            gexp = gp.tile([CO, 8], f32, tag="gexp")
            nc.scalar.activation(gexp[:, 0:5], zg[:, 0:5], AF.Exp)
            gs = gp.tile([CO, 1], f32, tag="gs")
            nc.vector.reduce_sum(gs[:], gexp[:, 0:5], axis=mybir.AxisListType.X)
            gr = gp.tile([CO, 1], f32, tag="gr")
            nc.vector.reciprocal(gr[:], gs[:])
            gv = gp.tile([CO, 8], f32, tag="gv")
            nc.vector.tensor_scalar_mul(gv[:, 0:5], gexp[:, 0:5], gr[:])
            nc.sync.dma_start(g_d[l][:, 0:5], gv[:, 0:5])
            gT2 = gp.tile([1, 5 * CO], f32, tag=f"gT{l}", name=f"gT{l}")
            nc.sync.dma_start(gT2[0:1, 0:5 * CO].rearrange("p (e c) -> p e c", c=CO),
                              g_d[l][:, 0:5].transpose([1, 0]))

            # broadcast g rows across all 128 partitions via ones outer-product
            gbt = []
            for e in range(5):
                pb = psg.tile([128, CO], f32, tag="pbb", name="pb")
                nc.tensor.matmul(pb[:], ones1[0:1, 0:128], gT2[0:1, e * CO:(e + 1) * CO],
                                 start=True, stop=True, skip_group_check=True)
                gbe = gp.tile([128, CO], f32, tag=f"gb{e}", name=f"gb{e}")
                nc.scalar.copy(gbe[:], pb[:])
                gbt.append(gbe)
            # mix experts chunk-wise, stage through DRAM, reload as f32r
            c0 = 0
            while c0 < COLS:
                cw = min(512, COLS - c0)
                acc = chp.tile([128, 512], f32, tag="acc")
                for e in range(5):
                    ech = chp.tile([128, 512], f32, tag="ech")
                    nc.sync.dma_start(ech[:, 0:cw], e_ap[e, :, c0:c0 + cw])
                    gb = gbt[e][:, 0:CO].unsqueeze(1).broadcast_to([128, cw // CO, CO])
                    ev = ech[:, 0:cw].rearrange("p (q c) -> p q c", c=CO)
                    av = acc[:, 0:cw].rearrange("p (q c) -> p q c", c=CO)
                    if e == 0:
                        nc.vector.tensor_tensor(av, ev, gb, op=ALU.mult)
                    else:
                        tmp = chp.tile([128, 512], f32, tag="tmp")
                        tv = tmp[:, 0:cw].rearrange("p (q c) -> p q c", c=CO)
                        nc.vector.tensor_tensor(tv, ev, gb, op=ALU.mult)
                        nc.vector.tensor_tensor(av, av, tv, op=ALU.add)
                nc.sync.dma_start(wsyn_d[l][:, c0:c0 + cw], acc[:, 0:cw])
                c0 += cw
            nc.sync.dma_start(wr_tile[:], wsyn_d[l][:, :].bitcast(f32r))

        w1r = wrp.tile([128, L1C], f32r, tag="w1r")
        w2r = wrp.tile([128, L2C], f32r, tag="w2r")
        wdr = wrp.tile([128, 4 * CO], f32r, tag="wdr")
        nc.sync.dma_start(wdr[:], wdn.bitcast(f32r))
        gate_and_synth(0, gw1, e1, L1C, w1r)
        gate_and_synth(1, gw2, e2, L2C, w2r)

        # ---------------- stats helpers ----------------------------------
        def stats_to_scales(lidx, st, nA, n_total, g_col, b_col):
            """Reduce bn_stats tile [64, nA*6] -> AllReduce -> scale/bias."""
            tmp = stp.tile([CO, 2 * max(nA, 1)], f32, tag="stmp")
            v = st[:, 0:nA * 6].rearrange("p (n s) -> p n s", s=6)
            # s1 = sum(cnt*mean) over even+odd halves
            nc.vector.tensor_tensor(tmp[:, 0:nA], v[:, :, 0], v[:, :, 1], op=ALU.mult)
            nc.vector.tensor_tensor(tmp[:, nA:2 * nA], v[:, :, 3], v[:, :, 4], op=ALU.mult)
            sums = stp.tile([CO, 2], f32, tag="sums")
            nc.vector.reduce_sum(sums[:, 0:1], tmp[:, 0:2 * nA], axis=mybir.AxisListType.X)
            # s2 = sum(cnt*mean^2 + cnt*var)
            nc.vector.tensor_tensor(tmp[:, 0:nA], v[:, :, 1], v[:, :, 1], op=ALU.mult)
            nc.vector.tensor_tensor(tmp[:, 0:nA], tmp[:, 0:nA], v[:, :, 0], op=ALU.mult)
            nc.vector.tensor_tensor(tmp[:, 0:nA], tmp[:, 0:nA], v[:, :, 2], op=ALU.add)
            nc.vector.tensor_tensor(tmp[:, nA:2 * nA], v[:, :, 4], v[:, :, 4], op=ALU.mult)
            nc.vector.tensor_tensor(tmp[:, nA:2 * nA], tmp[:, nA:2 * nA], v[:, :, 3], op=ALU.mult)
            nc.vector.tensor_tensor(tmp[:, nA:2 * nA], tmp[:, nA:2 * nA], v[:, :, 5], op=ALU.add)
            nc.vector.reduce_sum(sums[:, 1:2], tmp[:, 0:2 * nA], axis=mybir.AxisListType.X)
            nc.sync.dma_start(ar_in[lidx][:, :], sums[:])
            nc.gpsimd.collective_compute(
                "AllReduce", ALU.add, replica_groups=RG,
                ins=[ar_in[lidx][:, :]], outs=[ar_out[lidx][:, :]])
            tot = stp.tile([CO, 2], f32, tag="tot")
            nc.sync.dma_start(tot[:], ar_out[lidx][:, :])
            mu = stp.tile([CO, 1], f32, tag=f"mu{lidx}")
            nc.vector.tensor_scalar_mul(mu[:], tot[:, 0:1], 1.0 / n_total)
            e2t = stp.tile([CO, 1], f32, tag="e2t")
            nc.vector.tensor_scalar_mul(e2t[:], tot[:, 1:2], 1.0 / n_total)
            var = stp.tile([CO, 1], f32, tag="var")
            nc.vector.tensor_tensor(var[:], mu[:], mu[:], op=ALU.mult)
            nc.vector.tensor_tensor(var[:], e2t[:], var[:], op=ALU.subtract)
            nc.vector.tensor_scalar_add(var[:], var[:], EPS)
            sd = stp.tile([CO, 1], f32, tag="sd")
            nc.scalar.sqrt(sd[:], var[:])
            inv = stp.tile([CO, 1], f32, tag="inv")
            nc.vector.reciprocal(inv[:], sd[:])
            s_ch = stp.tile([CO, 1], f32, tag=f"s{lidx}")
            nc.vector.tensor_tensor(s_ch[:], inv[:], afft[:, g_col:g_col + 1], op=ALU.mult)
            negs = stp.tile([CO, 1], f32, tag="negs")
            nc.vector.tensor_scalar_mul(negs[:], s_ch[:], -1.0)
            b_ch = stp.tile([CO, 1], f32, tag=f"b{lidx}")
            nc.vector.scalar_tensor_tensor(b_ch[:], mu[:], negs[:],
                                           afft[:, b_col:b_col + 1],
                                           op0=ALU.mult, op1=ALU.add)
            return s_ch, b_ch

        # ---------------- conv layers ----------------------------------
        def conv_layer(l, wr, n_out, src_plane_ap, groups, y_planes,
                       st, owned_range):
            """l=0: Ci=32, kh-quads K=128 + 3-way row-tiled kh4 singles.
            l=1: Ci=64, kh-pairs K=128 + 2-way row-tiled kh4 singles."""
            ci = CI1 if l == 0 else CO
            nstk = 25 if l == 0 else 50
            splits = (9, 8, 8) if l == 0 else (13, 12)
            # singles assignment: (group, col-slot, kd, kw); bank 0 shares pP
            assign = []
            t = 0
            for g, cnt in enumerate(splits):
                for sg in range(cnt):
                    kd, kw = divmod(t, 5)
                    assign.append((g, sg, kd, kw))
                    t += 1
            # interleave groups for PE row-tile concurrency
            inter = []
            for sslot in range(max(splits)):
                for g in range(len(splits)):
                    if sslot < splits[g]:
                        inter.append(assign[sum(splits[:g]) + sslot])
            lastg = {}
            for i, (g, sg, kd, kw) in enumerate(inter):
                lastg[g] = i
            tiles = {}

            def ensure(p):
                if p in tiles:
                    return
                tpl = plane.tile([128, PS], f32r, tag="plane", name="xpl")
                for g in range(groups):
                    nc.sync.dma_start(
                        tpl[ci * g:ci * g + ci, 0:PS - g * WP],
                        src_plane_ap(p)[:, g * WP:PS].bitcast(f32r))
                tiles[p] = tpl

            iA = 0
            for dp in range(n_out):
                for p in range(dp, dp + 5) if dp == 0 else [dp + 4]:
                    ensure(p)
                for blk in range(nblk):
                    h0 = blk * 8
                    pP = ps.tile([128, NB], f32, tag="psA", name="pP")
                    pQ = ps.tile([128, NB], f32, tag="psB", name="pQ")
                    pR = (ps.tile([128, NB], f32, tag="psC", name="pR")
                          if l == 0 else None)
                    bankof = (pP, pQ, pR) if l == 0 else (pP, pQ)
                    # K=128 tap-stacked matmuls -> pP
                    for q in range(nstk):
                        if l == 0:
                            kd, kw = divmod(q, 5)
                            ro = h0
                        else:
                            kk, half = divmod(q, 2)
                            kd, kw = divmod(kk, 5)
                            ro = h0 + 2 * half
                        lhsT = wr[:, q * CO:(q + 1) * CO]
                        xv = tiles[dp + kd][:, :].rearrange("p (h w) -> p h w", w=WP)
                        nc.tensor.matmul(pP[0:64, :], lhsT,
                                         xv[:, ro:ro + 8, kw:kw + W],
                                         start=(q == 0), stop=False,
                                         tile_position=(0, 0), skip_group_check=True)
                    # kh=4 singles, row-tiled into per-group banks
                    for i, (g, sg, kd, kw) in enumerate(inter):
                        col = (nstk + sg) * CO
                        lhsT = wr[ci * g:ci * (g + 1), col:col + CO]
                        bank = bankof[g]
                        base_r = h0 + 4 - g
                        xv = tiles[dp + kd][ci * g:ci * (g + 1), :].rearrange(
                            "p (h w) -> p h w", w=WP)
                        nc.tensor.matmul(bank[0:64, :], lhsT,
                                         xv[:, base_r:base_r + 8, kw:kw + W],
                                         start=(g > 0 and sg == 0),
                                         stop=(i == lastg[g]),
                                         tile_position=(ci * g, 0),
                                         skip_group_check=True)
                    # merge banks + evacuate (one PSUM operand per DVE op)
                    ev = evp.tile([CO, NB], f32, tag="ev")
                    nc.scalar.copy(ev[:], pP[0:64, :])
                    nc.vector.tensor_tensor(ev[:], ev[:], pQ[0:64, :], op=ALU.add)
                    if l == 0:
                        nc.vector.tensor_tensor(ev[:], ev[:], pR[0:64, :],
                                                op=ALU.add)
                    nc.sync.dma_start(y_planes[dp][:, h0 * W:h0 * W + NB], ev[:])
                    if owned_range[0] <= dp < owned_range[1]:
                        nc.vector.bn_stats(st[:, iA * 6:(iA + 1) * 6], ev[:])
                        iA += 1
            return iA

        # ---------------- normalize + mish sweeps ----------------------
        def norm_mish(y_planes, out_fn, s_list, b_list, batch, half_cols, parts):
            """Two-sweep mish over plane list; s_list/b_list per plane."""
            n = len(y_planes)
            halves = [(p, c0) for p in range(n)
                      for c0 in range(0, y_planes[p].shape[1], half_cols)]
            for b0 in range(0, len(halves), batch):
                grp = halves[b0:b0 + batch]
                for (p, c0) in grp:   # sweep A: u=exp(y*s+b); w=ln(u+1)
                    yt = nop.tile([parts, half_cols], f32, tag="ni")
                    nc.sync.dma_start(yt[:], y_planes[p][0:parts, c0:c0 + half_cols])
                    ut = nop.tile([parts, half_cols], f32, tag="nu")
                    nc.scalar.activation(ut[:], yt[:], AF.Exp,
                                         bias=b_list[p][0:parts, :], scale=s_list[p][0:parts, :])
                    nc.vector.tensor_scalar_add(ut[:], ut[:], 1.0)
                    nc.scalar.activation(ut[:], ut[:], AF.Ln)
                    wdst = w_d[p] if parts == CO else w3_d[p]
                    nc.sync.dma_start(wdst[:, c0:c0 + half_cols][0:parts, :], ut[:])
                for (p, c0) in grp:   # sweep B: h = (y*s+b) * tanh(w)
                    wt = nop.tile([parts, half_cols], f32, tag="nu")
                    wsrc = w_d[p] if parts == CO else w3_d[p]
                    nc.sync.dma_start(wt[:], wsrc[:, c0:c0 + half_cols][0:parts, :])
                    nc.scalar.activation(wt[:], wt[:], AF.Tanh)
                    yt = nop.tile([parts, half_cols], f32, tag="ni")
                    nc.sync.dma_start(yt[:], y_planes[p][0:parts, c0:c0 + half_cols])
                    nc.vector.tensor_scalar(yt[:], yt[:], s_list[p][0:parts, :],
                                            b_list[p][0:parts, :],
                                            op0=ALU.mult, op1=ALU.add)
                    nc.gpsimd.tensor_tensor(yt[:], yt[:], wt[:], op=ALU.mult)
                    out_fn(p, c0, yt)

        # ================= layer 1 =================
        st1 = stp.tile([CO, max(slab * nblk, 1) * 6], f32, tag="st1")
        nc.vector.memset(st1[:], 0.0)
        nA1 = conv_layer(0, w1r, P1, lambda p: xin[:, p].rearrange("c h w -> c (h w)"),
                         4, y1_d, st1, (2, 2 + slab))
        s1, b1 = stats_to_scales(0, st1, nA1, N1, 0, 1)
        # halo-plane masked scale/bias (edge cores zero their out-of-range planes)
        mlo = maskt[:, 0:1]
        mhi = maskt[:, 1:2]
        s1lo = stp.tile([CO, 1], f32, tag="s1lo")
        b1lo = stp.tile([CO, 1], f32, tag="b1lo")
        s1hi = stp.tile([CO, 1], f32, tag="s1hi")
        b1hi = stp.tile([CO, 1], f32, tag="b1hi")
        nc.vector.tensor_tensor(s1lo[:], s1[:], mlo, op=ALU.mult)
        nc.vector.tensor_tensor(b1lo[:], b1[:], mlo, op=ALU.mult)
        nc.vector.tensor_tensor(s1hi[:], s1[:], mhi, op=ALU.mult)
        nc.vector.tensor_tensor(b1hi[:], b1[:], mhi, op=ALU.mult)
        s_l1 = [s1lo, s1lo] + [s1] * (P1 - 4) + [s1hi, s1hi]
        b_l1 = [b1lo, b1lo] + [b1] * (P1 - 4) + [b1hi, b1hi]

        def h_out(p, c0, yt):
            # write into hext interior: rows c0//W .. +half rows, cols 2..2+W
            r0 = c0 // W
            nrow = yt.shape[1] // W
            dst = hext[p][:, :].rearrange("c (h w) -> c h w", w=WP)
            dst = dst[:, 2 + r0:2 + r0 + nrow, 2:2 + W]
            nc.sync.dma_start(dst, yt[:].rearrange("c (h w) -> c h w", w=W))

        norm_mish(y1_d, h_out, s_l1, b_l1, batch=8, half_cols=H * W // 2, parts=CO)

        # ================= layer 2 =================
        st2 = stp.tile([CO, max(slab * nblk, 1) * 6], f32, tag="st2")
        nc.vector.memset(st2[:], 0.0)
        nA2 = conv_layer(1, w2r, slab, lambda p: hext[p][:, :],
                         2, y2_d, st2, (0, slab))
        s2, b2 = stats_to_scales(1, st2, nA2, N1, 2, 3)

        def xs_out(p, c0, yt):
            nc.sync.dma_start(xskip[:, p].rearrange("c h w -> c (h w)")[:, c0:c0 + yt.shape[1]], yt[:])

        norm_mish(y2_d, xs_out, [s2] * slab, [b2] * slab,
                  batch=8, half_cols=H * W // 2, parts=CO)

        # ================= down layer =================
        std = stp.tile([CO, max(DP * nblkd, 1) * 6], f32, tag="std")
        nc.vector.memset(std[:], 0.0)
        iD = 0
        for dd in range(DP):
            xdt = plane.tile([128, H * W], f32r, tag="plane", name="xdt")
            nc.sync.dma_start(xdt[0:64, :],
                              xskip[:, 2 * dd].rearrange("c h w -> c (h w)").bitcast(f32r))
            nc.sync.dma_start(xdt[64:128, :],
                              xskip[:, 2 * dd + 1].rearrange("c h w -> c (h w)").bitcast(f32r))
            xv = xdt[:, :].rearrange("p (h th w tw) -> p h th w tw", th=2, tw=2, w=WD)
            for blk in range(nblkd):
                r0 = blk * dbr
                pP = ps.tile([128, NBD], f32, tag="psA", name="pPd")
                for g in range(4):
                    kh, kw = divmod(g, 2)
                    lhsT = wdr[:, g * CO:(g + 1) * CO]
                    nc.tensor.matmul(pP[0:64, :], lhsT,
                                     xv[:, r0:r0 + dbr, kh, :, kw],
                                     start=(g == 0), stop=(g == 3),
                                     tile_position=(0, 0), skip_group_check=True)
                ev = evp.tile([CO, NBD], f32, tag="ev")
                nc.scalar.copy(ev[:], pP[0:64, :])
                nc.sync.dma_start(y3_d[dd][:, blk * NBD:(blk + 1) * NBD], ev[:])
                nc.vector.bn_stats(std[:, iD * 6:(iD + 1) * 6], ev[:])
                iD += 1
        sdn, bdn = stats_to_scales(2, std, iD, ND, 4, 5)

        def y_out(p, c0, yt):
            blk = c0 // NBD
            yv = yt[:].rearrange("c (h w) -> c h w", w=WD)
            nc.sync.dma_start(yout[:, p, blk * dbr:(blk + 1) * dbr, :], yv)

        norm_mish(y3_d, y_out, [sdn] * DP, [bdn] * DP,
                  batch=8, half_cols=NBD, parts=CO)

    nc.compile()
    return nc


# ----------------------------------------------------------------------------
# host entry
# ----------------------------------------------------------------------------

def make_core_inputs(cfg, x, t, layers, down_w, down_gamma, down_beta):
    """Per-core input dicts (8 cores = 2 samples x 4 slabs)."""
    D, H, W = cfg["D"], cfg["H"], cfg["W"]
    slab = D // 4
    (E1, gwt1, g1, b1), (E2, gwt2, g2, b2) = layers
    e1 = expert_lhsT_l1(E1)
    e2 = expert_lhsT_l2(E2)
    wd = down_lhsT(down_w)
    aff = np.stack([g1, b1, g2, b2, down_gamma, down_beta], axis=1).astype(np.float32)
    ins = []
    for core in range(8):
        n, s = divmod(core, 4)
        xp = np.pad(x[n], ((0, 0), (4, 4), (2, 2), (2, 2)))
        xin = np.ascontiguousarray(xp[:, slab * s: slab * s + slab + 8])
        ta = np.concatenate([t[n], [1.0]]).astype(np.float32).reshape(T_DIM + 1, 1)
        m = np.repeat(np.array([[0.0 if s == 0 else 1.0,
                                 0.0 if s == 3 else 1.0]], np.float32), 64, axis=0)
        ins.append({
            "xin": xin.astype(np.float32), "taug": ta,
            "e1": e1, "e2": e2, "gw1": gwt1, "gw2": gwt2,
            "wdn": wd, "aff": aff, "mask": m,
        })
    return ins


def prep_layers(inp):
    E1 = expert_stack(inp["l1_conv5"], inp["l1_conv3"], inp["l1_conv1"],
                      inp["l1_avg3"], inp["l1_avg5"])
    E2 = expert_stack(inp["l2_conv5"], inp["l2_conv3"], inp["l2_conv1"],
                      inp["l2_avg3"], inp["l2_avg5"])
    gw1 = gate_lhsT(inp["l1_gw"], inp["l1_gb"])
    gw2 = gate_lhsT(inp["l2_gw"], inp["l2_gb"])
    return ((E1, gw1, inp["l1_gamma"], inp["l1_beta"]),
            (E2, gw2, inp["l2_gamma"], inp["l2_beta"]))


_PROG_CACHE = {}
LAST_EXEC_NS = None


def kernel(**inputs):
    global LAST_EXEC_NS
    cfg = FULL
    D, H, W = cfg["D"], cfg["H"], cfg["W"]
    slab = D // 4
    inp = {k: np.asarray(v, dtype=np.float32) for k, v in inputs.items()}
    layers = prep_layers(inp)
    ins = make_core_inputs(cfg, inp["x"], inp["t"], layers,
                           inp["down_w"], inp["down_gamma"], inp["down_beta"])
    key = (D, H, W)
    if key not in _PROG_CACHE:
        _PROG_CACHE[key] = build_program(cfg)
    nc = _PROG_CACHE[key]
    trace = os.environ.get("BASS_KERNEL_PROFILE", "0") == "1"
    res = bass_utils.run_bass_kernel_spmd(nc, ins, core_ids=list(range(8)),
                                          trace=trace)
    LAST_EXEC_NS = res.exec_time_ns
    N = 2
    xskip = np.zeros((N, CO, D, H, W), np.float32)
    y = np.zeros((N, CO, D // 2, H // 2, W // 2), np.float32)
    for core in range(8):
        n, s = divmod(core, 4)
        r = res.results[core]
        xskip[n][:, slab * s:slab * (s + 1)] = r["xskip"].reshape(CO, slab, H, W)
        y[n][:, (slab // 2) * s:(slab // 2) * (s + 1)] = \
            r["yout"].reshape(CO, slab // 2, H // 2, W // 2)
    return (y, xskip)


# revision 14
# speedup vs baseline: 5370.6217x; 5370.6217x over previous
"""Trainium2 Bass kernel for nn_MoDEEncoderBlock (MoDE encoder block).

Sharding: 8 cores = 2 samples x 4 D-slabs (16 output planes each).
Each core runs: mode_conv1 (with 2-plane halo recompute) -> instance_norm+mish
-> mode_conv2 -> instance_norm+mish (= x_skip) -> strided down conv ->
instance_norm+mish (= y). Instance-norm statistics are globalized with a tiny
AllReduce among each sample's 4 cores.

Conv = 125 shifted matmuls accumulating in PSUM, K-stacked (layer1: 4 kh-shifted
copies x 32ci = K128 quads; layer2: 2 copies x 64ci = K128 pairs), with two
512-position output blocks running concurrently via PE column tiling. Matmuls
use float32r (full-rate fp32).
"""
import os
import sys

for _p in ("/opt/trn_rl_repo", "/root/.axon_site/_ro/trn_rl_repo"):
    if os.path.isdir(_p) and _p not in sys.path:
        sys.path.insert(0, _p)

import numpy as np
import concourse.bacc as bacc
import concourse.mybir as mybir
import concourse.tile as tile
from concourse import bass_utils

f32 = mybir.dt.float32
f32r = mybir.dt.float32r
AF = mybir.ActivationFunctionType
ALU = mybir.AluOpType

EPS = 1e-5
CO = 64
CI1 = 32
T_DIM = 10
RG = [[0, 1, 2, 3], [4, 5, 6, 7]]

FULL = dict(D=64, H=64, W=64)
MINI = dict(D=16, H=16, W=16)


# ----------------------------------------------------------------------------
# host-side weight preparation (pure layout/static-weight transforms)
# ----------------------------------------------------------------------------

def _pad_k(k, p):
    return np.pad(k, ((0, 0), (0, 0), (p, p), (p, p), (p, p)))


def expert_stack(c5, c3, c1, a3, a5):
    """(5, Co, Ci, 5, 5, 5) expert kernel stack, mirrors reference."""
    pool3 = np.full((3, 3, 3), np.float32(1.0) / np.float32(27.0), np.float32)
    pool5 = np.full((5, 5, 5), np.float32(1.0) / np.float32(125.0), np.float32)
    return np.stack([
        c5,
        _pad_k(c3, 1),
        _pad_k(c1, 2),
        _pad_k(a3 * pool3, 1),
        a5 * pool5,
    ], axis=0).astype(np.float32)


def expert_lhsT_l1(E):
    """(5, 128, 34*64): kh0-3 quads (partition-stacked) + kh=4 singles
    packed 3-per-column on row-groups 0/32/64."""
    quad = E[:, :, :, :, 0:4, :]                       # e co ci kd j kw
    quad = quad.transpose(0, 4, 2, 3, 5, 1)            # e j ci kd kw co
    quad = quad.reshape(5, 4 * CI1, 25 * CO)
    sing = E[:, :, :, :, 4, :]                         # e co ci kd kw
    sing = sing.transpose(0, 2, 3, 4, 1).reshape(5, CI1, 25, CO)
    sing_f = np.zeros((5, 128, 9 * CO), np.float32)
    for i, (g, sl) in enumerate(((0, slice(0, 9)), (1, slice(9, 17)),
                                 (2, slice(17, 25)))):
        cnt = sl.stop - sl.start
        sing_f[:, 32 * g:32 * g + CI1, 0:cnt * CO] = \
            sing[:, :, sl].reshape(5, CI1, cnt * CO)
    return np.concatenate([quad, sing_f], axis=2).astype(np.float32)


def expert_lhsT_l2(E):
    """(5, 128, 63*64): kh-pairs (2-stacked) + kh=4 singles packed
    2-per-column on row-groups 0/64 (13 + 12)."""
    ci = CO
    pair = E[:, :, :, :, 0:4, :].reshape(5, CO, ci, 5, 2, 2, 5)  # e co ci kd half j kw
    pair = pair.transpose(0, 5, 2, 3, 6, 4, 1)                   # e j ci kd kw half co
    pair = pair.reshape(5, 2 * ci, 50 * CO)
    sing = E[:, :, :, :, 4, :].transpose(0, 2, 3, 4, 1).reshape(5, ci, 25, CO)
    sing_f = np.zeros((5, 128, 13 * CO), np.float32)
    sing_f[:, 0:ci, 0:13 * CO] = sing[:, :, 0:13].reshape(5, ci, 13 * CO)
    sing_f[:, ci:2 * ci, 0:12 * CO] = sing[:, :, 13:25].reshape(5, ci, 12 * CO)
    return np.concatenate([pair, sing_f], axis=2).astype(np.float32)


def down_lhsT(dw):
    """(128, 4*64): kd-pairs stacked; col (kh*2+kw)*64+co."""
    # dw: (Co, Ci=64, 2, 2, 2)
    a = dw.transpose(2, 1, 3, 4, 0)          # kd ci kh kw co
    return a.reshape(128, 4 * CO).astype(np.float32)


def gate_lhsT(gw, gb):
    """(11, 5*64): row k<10 = gw[:, k]; row 10 = gb."""
    out = np.zeros((T_DIM + 1, 5 * CO), np.float32)
    out[:T_DIM] = gw.T
    out[T_DIM] = gb
    return out


# ----------------------------------------------------------------------------
# device program
# ----------------------------------------------------------------------------

def build_program(cfg):
    D, H, W = cfg["D"], cfg["H"], cfg["W"]
    slab = D // 4
    HP, WP = H + 4, W + 4
    PS = HP * WP
    P1 = slab + 4            # conv1 output planes (incl 2+2 halo)
    PX = slab + 8            # x input planes
    nblk = H // 8            # 8-row blocks per plane
    npair = nblk // 2
    NB = 8 * W               # psum free size per block
    HD, WD = H // 2, W // 2
    dbr = min(HD, 512 // WD)     # down block rows
    nblkd = HD // dbr            # blocks per down plane (2 full, 1 mini)
    DP = slab // 2
    NBD = dbr * WD
    N1 = D * H * W
    ND = (D // 2) * (H // 2) * (W // 2)
    L1C = 34 * CO
    L2C = 63 * CO

    nc = bacc.Bacc("TRN2", target_bir_lowering=False, debug=False,
                   enable_asserts=False, num_devices=8)

    # ---- I/O ----
    xin = nc.dram_tensor("xin", [CI1, PX, HP, WP], f32, kind="ExternalInput").ap()
    taug = nc.dram_tensor("taug", [T_DIM + 1, 1], f32, kind="ExternalInput").ap()
    e1 = nc.dram_tensor("e1", [5, 128, L1C], f32, kind="ExternalInput").ap()
    e2 = nc.dram_tensor("e2", [5, 128, L2C], f32, kind="ExternalInput").ap()
    gw1 = nc.dram_tensor("gw1", [T_DIM + 1, 5 * CO], f32, kind="ExternalInput").ap()
    gw2 = nc.dram_tensor("gw2", [T_DIM + 1, 5 * CO], f32, kind="ExternalInput").ap()
    wdn = nc.dram_tensor("wdn", [128, 4 * CO], f32, kind="ExternalInput").ap()
    aff = nc.dram_tensor("aff", [CO, 6], f32, kind="ExternalInput").ap()
    mask = nc.dram_tensor("mask", [CO, 2], f32, kind="ExternalInput").ap()
    xskip = nc.dram_tensor("xskip", [CO, slab, H, W], f32, kind="ExternalOutput").ap()
    yout = nc.dram_tensor("yout", [CO, DP, HD, WD], f32, kind="ExternalOutput").ap()

    from contextlib import ExitStack
    with tile.TileContext(nc) as tc, ExitStack() as ctx:
        plane = ctx.enter_context(tc.tile_pool(name="plane", bufs=5))
        wrp = ctx.enter_context(tc.tile_pool(name="wrp", bufs=1))
        chp = ctx.enter_context(tc.tile_pool(name="chp", bufs=2))
        evp = ctx.enter_context(tc.tile_pool(name="evp", bufs=3))
        stp = ctx.enter_context(tc.tile_pool(name="stp", bufs=1))
        nop = ctx.enter_context(tc.tile_pool(name="nop", bufs=3))
        gp = ctx.enter_context(tc.tile_pool(name="gp", bufs=1))
        ps = ctx.enter_context(tc.tile_pool(name="ps", bufs=2, space="PSUM"))
        psg = ctx.enter_context(tc.tile_pool(name="psg", bufs=1, space="PSUM"))
        dr = ctx.enter_context(tc.tile_pool(name="dr", bufs=1, space="DRAM"))

        # ---- DRAM scratch ----
        y1_d = [dr.tile([CO, H * W], f32, tag=f"y1_{p}", name=f"y1_{p}") for p in range(P1)]
        hext = [dr.tile([CO, PS], f32, tag=f"hx_{p}", name=f"hx_{p}") for p in range(P1)]
        y2_d = [dr.tile([CO, H * W], f32, tag=f"y2_{p}", name=f"y2_{p}") for p in range(slab)]
        y3_d = [dr.tile([CO, nblkd * NBD], f32, tag=f"y3_{p}", name=f"y3_{p}") for p in range(DP)]
        w_d = [dr.tile([CO, H * W], f32, tag=f"wd_{p}", name=f"wd_{p}") for p in range(P1)]
        w3_d = [dr.tile([CO, nblkd * NBD], f32, tag=f"w3_{p}", name=f"w3_{p}") for p in range(DP)]
        g_d = [dr.tile([CO, 8], f32, tag=f"g_{l}", name=f"g_{l}") for l in range(2)]
        wsyn_d = [dr.tile([128, L1C], f32, tag="ws1", name="ws1"),
                  dr.tile([128, L2C], f32, tag="ws2", name="ws2")]
        ar_in = [dr.tile([CO, 2], f32, tag=f"ari{l}", name=f"ari{l}") for l in range(3)]
        ar_out = [dr.tile([CO, 2], f32, tag=f"aro{l}", name=f"aro{l}")
                  for l in range(3)]

        # ---- constants / small tiles ----
        afft = gp.tile([CO, 6], f32, tag="afft")
        nc.sync.dma_start(afft[:], aff)
        maskt = gp.tile([CO, 2], f32, tag="maskt")
        nc.sync.dma_start(maskt[:], mask)
        taut = gp.tile([T_DIM + 1, 1], f32, tag="taut")
        nc.sync.dma_start(taut[:], taug)
        ones1 = gp.tile([1, 128], f32, tag="ones1")
        nc.vector.memset(ones1[:], 1.0)

        # zero h_ext padding (borders). Use a zero tile, DMA over all hext planes.
        zt = nop.tile([CO, 2048], f32, tag="ni")
        nc.vector.memset(zt[:], 0.0)
        for p in range(P1):
            c0 = 0
            while c0 < PS:
                cw = min(2048, PS - c0)
                nc.sync.dma_start(hext[p][:, c0:c0 + cw], zt[:, 0:cw])
                c0 += cw

        # ---------------- gate + weight synthesis (per mode_conv layer) -------
        def gate_and_synth(l, gw_ap, e_ap, COLS, wr_tile):
            gwt = gp.tile([T_DIM + 1, 5 * CO], f32, tag="gwt")
            nc.sync.dma_start(gwt[:], gw_ap)
            zg = psg.tile([CO, 512], f32, tag="zg")
            for e in range(5):
                nc.tensor.matmul(zg[:, e:e + 1], gwt[0:T_DIM + 1, e * CO:(e + 1) * CO],
                                 taut[0:T_DIM + 1, 0:1],
                                 start=(e == 0), stop=(e == 4), skip_group_check=True)
            gexp = gp.tile([CO, 8], f32, tag="gexp")
            nc.scalar.activation(gexp[:, 0:5], zg[:, 0:5], AF.Exp)
            gs = gp.tile([CO, 1], f32, tag="gs")
            nc.vector.reduce_sum(gs[:], gexp[:, 0:5], axis=mybir.AxisListType.X)
            gr = gp.tile([CO, 1], f32, tag="gr")
            nc.vector.reciprocal(gr[:], gs[:])
            gv = gp.tile([CO, 8], f32, tag="gv")
            nc.vector.tensor_scalar_mul(gv[:, 0:5], gexp[:, 0:5], gr[:])
            nc.sync.dma_start(g_d[l][:, 0:5], gv[:, 0:5])
            gT2 = gp.tile([1, 5 * CO], f32, tag=f"gT{l}", name=f"gT{l}")
            nc.sync.dma_start(gT2[0:1, 0:5 * CO].rearrange("p (e c) -> p e c", c=CO),
                              g_d[l][:, 0:5].transpose([1, 0]))

            # broadcast g rows across all 128 partitions via ones outer-product
            gbt = []
            for e in range(5):
                pb = psg.tile([128, CO], f32, tag="pbb", name="pb")
                nc.tensor.matmul(pb[:], ones1[0:1, 0:128], gT2[0:1, e * CO:(e + 1) * CO],
                                 start=True, stop=True, skip_group_check=True)
                gbe = gp.tile([128, CO], f32, tag=f"gb{e}", name=f"gb{e}")
                nc.scalar.copy(gbe[:], pb[:])
                gbt.append(gbe)
            # mix experts chunk-wise, stage through DRAM, reload as f32r
            c0 = 0
            while c0 < COLS:
                cw = min(512, COLS - c0)
                acc = chp.tile([128, 512], f32, tag="acc")
                for e in range(5):
                    ech = chp.tile([128, 512], f32, tag="ech")
                    nc.sync.dma_start(ech[:, 0:cw], e_ap[e, :, c0:c0 + cw])
                    gb = gbt[e][:, 0:CO].unsqueeze(1).broadcast_to([128, cw // CO, CO])
                    ev = ech[:, 0:cw].rearrange("p (q c) -> p q c", c=CO)
                    av = acc[:, 0:cw].rearrange("p (q c) -> p q c", c=CO)
                    if e == 0:
                        nc.vector.tensor_tensor(av, ev, gb, op=ALU.mult)
                    else:
                        tmp = chp.tile([128, 512], f32, tag="tmp")
                        tv = tmp[:, 0:cw].rearrange("p (q c) -> p q c", c=CO)
                        nc.vector.tensor_tensor(tv, ev, gb, op=ALU.mult)
                        nc.vector.tensor_tensor(av, av, tv, op=ALU.add)
                nc.sync.dma_start(wsyn_d[l][:, c0:c0 + cw], acc[:, 0:cw])
                c0 += cw
            nc.sync.dma_start(wr_tile[:], wsyn_d[l][:, :].bitcast(f32r))

        w1r = wrp.tile([128, L1C], f32r, tag="w1r")
        w2r = wrp.tile([128, L2C], f32r, tag="w2r")
        wdr = wrp.tile([128, 4 * CO], f32r, tag="wdr")
        nc.sync.dma_start(wdr[:], wdn.bitcast(f32r))
        gate_and_synth(0, gw1, e1, L1C, w1r)
        gate_and_synth(1, gw2, e2, L2C, w2r)

        # ---------------- stats helpers ----------------------------------
        def stats_to_scales(lidx, st, nA, n_total, g_col, b_col):
            """Reduce bn_stats tile [64, nA*6] -> AllReduce -> scale/bias."""
            tmp = stp.tile([CO, 2 * max(nA, 1)], f32, tag="stmp")
            v = st[:, 0:nA * 6].rearrange("p (n s) -> p n s", s=6)
            # s1 = sum(cnt*mean) over even+odd halves
            nc.vector.tensor_tensor(tmp[:, 0:nA], v[:, :, 0], v[:, :, 1], op=ALU.mult)
            nc.vector.tensor_tensor(tmp[:, nA:2 * nA], v[:, :, 3], v[:, :, 4], op=ALU.mult)
            sums = stp.tile([CO, 2], f32, tag="sums")
            nc.vector.reduce_sum(sums[:, 0:1], tmp[:, 0:2 * nA], axis=mybir.AxisListType.X)
            # s2 = sum(cnt*mean^2 + cnt*var)
            nc.vector.tensor_tensor(tmp[:, 0:nA], v[:, :, 1], v[:, :, 1], op=ALU.mult)
            nc.vector.tensor_tensor(tmp[:, 0:nA], tmp[:, 0:nA], v[:, :, 0], op=ALU.mult)
            nc.vector.tensor_tensor(tmp[:, 0:nA], tmp[:, 0:nA], v[:, :, 2], op=ALU.add)
            nc.vector.tensor_tensor(tmp[:, nA:2 * nA], v[:, :, 4], v[:, :, 4], op=ALU.mult)
            nc.vector.tensor_tensor(tmp[:, nA:2 * nA], tmp[:, nA:2 * nA], v[:, :, 3], op=ALU.mult)
            nc.vector.tensor_tensor(tmp[:, nA:2 * nA], tmp[:, nA:2 * nA], v[:, :, 5], op=ALU.add)
            nc.vector.reduce_sum(sums[:, 1:2], tmp[:, 0:2 * nA], axis=mybir.AxisListType.X)
            nc.sync.dma_start(ar_in[lidx][:, :], sums[:])
            nc.gpsimd.collective_compute(
                "AllReduce", ALU.add, replica_groups=RG,
                ins=[ar_in[lidx][:, :]], outs=[ar_out[lidx][:, :]])
            tot = stp.tile([CO, 2], f32, tag="tot")
            nc.sync.dma_start(tot[:], ar_out[lidx][:, :])
            mu = stp.tile([CO, 1], f32, tag=f"mu{lidx}")
            nc.vector.tensor_scalar_mul(mu[:], tot[:, 0:1], 1.0 / n_total)
            e2t = stp.tile([CO, 1], f32, tag="e2t")
            nc.vector.tensor_scalar_mul(e2t[:], tot[:, 1:2], 1.0 / n_total)
            var = stp.tile([CO, 1], f32, tag="var")
            nc.vector.tensor_tensor(var[:], mu[:], mu[:], op=ALU.mult)
            nc.vector.tensor_tensor(var[:], e2t[:], var[:], op=ALU.subtract)
            nc.vector.tensor_scalar_add(var[:], var[:], EPS)
            sd = stp.tile([CO, 1], f32, tag="sd")
            nc.scalar.sqrt(sd[:], var[:])
            inv = stp.tile([CO, 1], f32, tag="inv")
            nc.vector.reciprocal(inv[:], sd[:])
            s_ch = stp.tile([CO, 1], f32, tag=f"s{lidx}")
            nc.vector.tensor_tensor(s_ch[:], inv[:], afft[:, g_col:g_col + 1], op=ALU.mult)
            negs = stp.tile([CO, 1], f32, tag="negs")
            nc.vector.tensor_scalar_mul(negs[:], s_ch[:], -1.0)
            b_ch = stp.tile([CO, 1], f32, tag=f"b{lidx}")
            nc.vector.scalar_tensor_tensor(b_ch[:], mu[:], negs[:],
                                           afft[:, b_col:b_col + 1],
                                           op0=ALU.mult, op1=ALU.add)
            return s_ch, b_ch

        # ---------------- conv layers ----------------------------------
        def conv_layer(l, wr, n_out, src_plane_ap, groups, y_planes,
                       st, owned_range):
            """l=0: Ci=32, kh-quads K=128 + 3-way row-tiled kh4 singles.
            l=1: Ci=64, kh-pairs K=128 + 2-way row-tiled kh4 singles."""
            ci = CI1 if l == 0 else CO
            nstk = 25 if l == 0 else 50
            splits = (9, 8, 8) if l == 0 else (13, 12)
            # singles assignment: (group, col-slot, kd, kw); bank 0 shares pP
            assign = []
            t = 0
            for g, cnt in enumerate(splits):
                for sg in range(cnt):
                    kd, kw = divmod(t, 5)
                    assign.append((g, sg, kd, kw))
                    t += 1
            # interleave groups for PE row-tile concurrency
            inter = []
            for sslot in range(max(splits)):
                for g in range(len(splits)):
                    if sslot < splits[g]:
                        inter.append(assign[sum(splits[:g]) + sslot])
            lastg = {}
            for i, (g, sg, kd, kw) in enumerate(inter):
                lastg[g] = i
            tiles = {}

            def ensure(p):
                if p in tiles:
                    return
                tpl = plane.tile([128, PS], f32r, tag="plane", name="xpl")
                for g in range(groups):
                    nc.sync.dma_start(
                        tpl[ci * g:ci * g + ci, 0:PS - g * WP],
                        src_plane_ap(p)[:, g * WP:PS].bitcast(f32r))
                tiles[p] = tpl

            iA = 0
            for dp in range(n_out):
                for p in range(dp, dp + 5) if dp == 0 else [dp + 4]:
                    ensure(p)
                for blk in range(nblk):
                    h0 = blk * 8
                    pP = ps.tile([128, NB], f32, tag="psA", name="pP")
                    pQ = ps.tile([128, NB], f32, tag="psB", name="pQ")
                    pR = (ps.tile([128, NB], f32, tag="psC", name="pR")
                          if l == 0 else None)
                    bankof = (pP, pQ, pR) if l == 0 else (pP, pQ)
                    # K=128 tap-stacked matmuls -> pP
                    for q in range(nstk):
                        if l == 0:
                            kd, kw = divmod(q, 5)
                            ro = h0
                        else:
                            kk, half = divmod(q, 2)
                            kd, kw = divmod(kk, 5)
                            ro = h0 + 2 * half
                        lhsT = wr[:, q * CO:(q + 1) * CO]
                        xv = tiles[dp + kd][:, :].rearrange("p (h w) -> p h w", w=WP)
                        nc.tensor.matmul(pP[0:64, :], lhsT,
                                         xv[:, ro:ro + 8, kw:kw + W],
                                         start=(q == 0), stop=False,
                                         tile_position=(0, 0), skip_group_check=True)
                    # kh=4 singles, row-tiled into per-group banks
                    for i, (g, sg, kd, kw) in enumerate(inter):
                        col = (nstk + sg) * CO
                        lhsT = wr[ci * g:ci * (g + 1), col:col + CO]
                        bank = bankof[g]
                        base_r = h0 + 4 - g
                        xv = tiles[dp + kd][ci * g:ci * (g + 1), :].rearrange(
                            "p (h w) -> p h w", w=WP)
                        nc.tensor.matmul(bank[0:64, :], lhsT,
                                         xv[:, base_r:base_r + 8, kw:kw + W],
                                         start=(g > 0 and sg == 0),
                                         stop=(i == lastg[g]),
                                         tile_position=(ci * g, 0),
                                         skip_group_check=True)
                    # merge banks + evacuate (one PSUM operand per DVE op)
                    ev = evp.tile([CO, NB], f32, tag="ev")
                    nc.scalar.copy(ev[:], pP[0:64, :])
                    nc.vector.tensor_tensor(ev[:], ev[:], pQ[0:64, :], op=ALU.add)
                    if l == 0:
                        nc.vector.tensor_tensor(ev[:], ev[:], pR[0:64, :],
                                                op=ALU.add)
                    nc.sync.dma_start(y_planes[dp][:, h0 * W:h0 * W + NB], ev[:])
                    if owned_range[0] <= dp < owned_range[1]:
                        nc.vector.bn_stats(st[:, iA * 6:(iA + 1) * 6], ev[:])
                        iA += 1
            return iA

        # ---------------- normalize + mish sweeps ----------------------
        def norm_mish(y_planes, out_fn, s_list, b_list, batch, half_cols, parts):
            """Two-sweep mish over plane list; s_list/b_list per plane."""
            n = len(y_planes)
            halves = [(p, c0) for p in range(n)
                      for c0 in range(0, y_planes[p].shape[1], half_cols)]
            for b0 in range(0, len(halves), batch):
                grp = halves[b0:b0 + batch]
                for (p, c0) in grp:   # sweep A: u=exp(y*s+b); w=ln(u+1)
                    yt = nop.tile([parts, half_cols], f32, tag="ni")
                    nc.sync.dma_start(yt[:], y_planes[p][0:parts, c0:c0 + half_cols])
                    ut = nop.tile([parts, half_cols], f32, tag="nu")
                    nc.scalar.activation(ut[:], yt[:], AF.Exp,
                                         bias=b_list[p][0:parts, :], scale=s_list[p][0:parts, :])
                    nc.vector.tensor_scalar_add(ut[:], ut[:], 1.0)
                    nc.scalar.activation(ut[:], ut[:], AF.Ln)
                    wdst = w_d[p] if parts == CO else w3_d[p]
                    nc.sync.dma_start(wdst[:, c0:c0 + half_cols][0:parts, :], ut[:])
                for (p, c0) in grp:   # sweep B: h = (y*s+b) * tanh(w)
                    wt = nop.tile([parts, half_cols], f32, tag="nu")
                    wsrc = w_d[p] if parts == CO else w3_d[p]
                    nc.sync.dma_start(wt[:], wsrc[:, c0:c0 + half_cols][0:parts, :])
                    nc.scalar.activation(wt[:], wt[:], AF.Tanh)
                    yt = nop.tile([parts, half_cols], f32, tag="ni")
                    nc.sync.dma_start(yt[:], y_planes[p][0:parts, c0:c0 + half_cols])
                    nc.vector.tensor_scalar(yt[:], yt[:], s_list[p][0:parts, :],
                                            b_list[p][0:parts, :],
                                            op0=ALU.mult, op1=ALU.add)
                    nc.gpsimd.tensor_tensor(yt[:], yt[:], wt[:], op=ALU.mult)
                    out_fn(p, c0, yt)

        # ================= layer 1 =================
        st1 = stp.tile([CO, max(slab * nblk, 1) * 6], f32, tag="st1")
        nc.vector.memset(st1[:], 0.0)
        nA1 = conv_layer(0, w1r, P1, lambda p: xin[:, p].rearrange("c h w -> c (h w)"),
                         4, y1_d, st1, (2, 2 + slab))
        s1, b1 = stats_to_scales(0, st1, nA1, N1, 0, 1)
        # halo-plane masked scale/bias (edge cores zero their out-of-range planes)
        mlo = maskt[:, 0:1]
        mhi = maskt[:, 1:2]
        s1lo = stp.tile([CO, 1], f32, tag="s1lo")
        b1lo = stp.tile([CO, 1], f32, tag="b1lo")
        s1hi = stp.tile([CO, 1], f32, tag="s1hi")
        b1hi = stp.tile([CO, 1], f32, tag="b1hi")
        nc.vector.tensor_tensor(s1lo[:], s1[:], mlo, op=ALU.mult)
        nc.vector.tensor_tensor(b1lo[:], b1[:], mlo, op=ALU.mult)
        nc.vector.tensor_tensor(s1hi[:], s1[:], mhi, op=ALU.mult)
        nc.vector.tensor_tensor(b1hi[:], b1[:], mhi, op=ALU.mult)
        s_l1 = [s1lo, s1lo] + [s1] * (P1 - 4) + [s1hi, s1hi]
        b_l1 = [b1lo, b1lo] + [b1] * (P1 - 4) + [b1hi, b1hi]

        def h_out(p, c0, yt):
            # write into hext interior: rows c0//W .. +half rows, cols 2..2+W
            r0 = c0 // W
            nrow = yt.shape[1] // W
            dst = hext[p][:, :].rearrange("c (h w) -> c h w", w=WP)
            dst = dst[:, 2 + r0:2 + r0 + nrow, 2:2 + W]
            nc.sync.dma_start(dst, yt[:].rearrange("c (h w) -> c h w", w=W))

        norm_mish(y1_d, h_out, s_l1, b_l1, batch=8, half_cols=H * W // 2, parts=CO)

        # ================= layer 2 =================
        st2 = stp.tile([CO, max(slab * nblk, 1) * 6], f32, tag="st2")
        nc.vector.memset(st2[:], 0.0)
        nA2 = conv_layer(1, w2r, slab, lambda p: hext[p][:, :],
                         2, y2_d, st2, (0, slab))
        s2, b2 = stats_to_scales(1, st2, nA2, N1, 2, 3)

        def xs_out(p, c0, yt):
            nc.sync.dma_start(xskip[:, p].rearrange("c h w -> c (h w)")[:, c0:c0 + yt.shape[1]], yt[:])

        norm_mish(y2_d, xs_out, [s2] * slab, [b2] * slab,
                  batch=8, half_cols=H * W // 2, parts=CO)

        # ================= down layer =================
        std = stp.tile([CO, max(DP * nblkd, 1) * 6], f32, tag="std")
        nc.vector.memset(std[:], 0.0)
        iD = 0
        for dd in range(DP):
            xdt = plane.tile([128, H * W], f32r, tag="plane", name="xdt")
            nc.sync.dma_start(xdt[0:64, :],
                              xskip[:, 2 * dd].rearrange("c h w -> c (h w)").bitcast(f32r))
            nc.sync.dma_start(xdt[64:128, :],
                              xskip[:, 2 * dd + 1].rearrange("c h w -> c (h w)").bitcast(f32r))
            xv = xdt[:, :].rearrange("p (h th w tw) -> p h th w tw", th=2, tw=2, w=WD)
            for blk in range(nblkd):
                r0 = blk * dbr
                pP = ps.tile([128, NBD], f32, tag="psA", name="pPd")
                for g in range(4):
                    kh, kw = divmod(g, 2)
                    lhsT = wdr[:, g * CO:(g + 1) * CO]
                    nc.tensor.matmul(pP[0:64, :], lhsT,
                                     xv[:, r0:r0 + dbr, kh, :, kw],
                                     start=(g == 0), stop=(g == 3),
                                     tile_position=(0, 0), skip_group_check=True)
                ev = evp.tile([CO, NBD], f32, tag="ev")
                nc.scalar.copy(ev[:], pP[0:64, :])
                nc.sync.dma_start(y3_d[dd][:, blk * NBD:(blk + 1) * NBD], ev[:])
                nc.vector.bn_stats(std[:, iD * 6:(iD + 1) * 6], ev[:])
                iD += 1
        sdn, bdn = stats_to_scales(2, std, iD, ND, 4, 5)

        def y_out(p, c0, yt):
            blk = c0 // NBD
            yv = yt[:].rearrange("c (h w) -> c h w", w=WD)
            nc.sync.dma_start(yout[:, p, blk * dbr:(blk + 1) * dbr, :], yv)

        norm_mish(y3_d, y_out, [sdn] * DP, [bdn] * DP,
                  batch=8, half_cols=NBD, parts=CO)

    nc.compile()
    return nc


# ----------------------------------------------------------------------------
# host entry
# ----------------------------------------------------------------------------

def make_core_inputs(cfg, x, t, layers, down_w, down_gamma, down_beta):
    """Per-core input dicts (8 cores = 2 samples x 4 slabs)."""
    D, H, W = cfg["D"], cfg["H"], cfg["W"]
    slab = D // 4
    (E1, gwt1, g1, b1), (E2, gwt2, g2, b2) = layers
    e1 = expert_lhsT_l1(E1)
    e2 = expert_lhsT_l2(E2)
    wd = down_lhsT(down_w)
    aff = np.stack([g1, b1, g2, b2, down_gamma, down_beta], axis=1).astype(np.float32)
    ins = []
    for core in range(8):
        n, s = divmod(core, 4)
        xp = np.pad(x[n], ((0, 0), (4, 4), (2, 2), (2, 2)))
        xin = np.ascontiguousarray(xp[:, slab * s: slab * s + slab + 8])
        ta = np.concatenate([t[n], [1.0]]).astype(np.float32).reshape(T_DIM + 1, 1)
        m = np.repeat(np.array([[0.0 if s == 0 else 1.0,
                                 0.0 if s == 3 else 1.0]], np.float32), 64, axis=0)
        ins.append({
            "xin": xin.astype(np.float32), "taug": ta,
            "e1": e1, "e2": e2, "gw1": gwt1, "gw2": gwt2,
            "wdn": wd, "aff": aff, "mask": m,
        })
    return ins


def prep_layers(inp):
    E1 = expert_stack(inp["l1_conv5"], inp["l1_conv3"], inp["l1_conv1"],
                      inp["l1_avg3"], inp["l1_avg5"])
    E2 = expert_stack(inp["l2_conv5"], inp["l2_conv3"], inp["l2_conv1"],
                      inp["l2_avg3"], inp["l2_avg5"])
    gw1 = gate_lhsT(inp["l1_gw"], inp["l1_gb"])
    gw2 = gate_lhsT(inp["l2_gw"], inp["l2_gb"])
    return ((E1, gw1, inp["l1_gamma"], inp["l1_beta"]),
            (E2, gw2, inp["l2_gamma"], inp["l2_beta"]))


_PROG_CACHE = {}
LAST_EXEC_NS = None


def kernel(**inputs):
    global LAST_EXEC_NS
    cfg = FULL
    D, H, W = cfg["D"], cfg["H"], cfg["W"]
    slab = D // 4
    inp = {k: np.asarray(v, dtype=np.float32) for k, v in inputs.items()}
    layers = prep_layers(inp)
    ins = make_core_inputs(cfg, inp["x"], inp["t"], layers,
                           inp["down_w"], inp["down_gamma"], inp["down_beta"])
    key = (D, H, W)
    if key not in _PROG_CACHE:
        _PROG_CACHE[key] = build_program(cfg)
    nc = _PROG_CACHE[key]
    trace = os.environ.get("BASS_KERNEL_PROFILE", "0") == "1"
    res = bass_utils.run_bass_kernel_spmd(nc, ins, core_ids=list(range(8)),
                                          trace=trace)
    LAST_EXEC_NS = res.exec_time_ns
    N = 2
    xskip = np.zeros((N, CO, D, H, W), np.float32)
    y = np.zeros((N, CO, D // 2, H // 2, W // 2), np.float32)
    for core in range(8):
        n, s = divmod(core, 4)
        r = res.results[core]
        xskip[n][:, slab * s:slab * (s + 1)] = r["xskip"].reshape(CO, slab, H, W)
        y[n][:, (slab // 2) * s:(slab // 2) * (s + 1)] = \
            r["yout"].reshape(CO, slab // 2, H // 2, W // 2)
    return (y, xskip)


def _make_runner(nc, n_cores):
    """Persistent compiled runner (mirrors bass2jax.run_bass_via_pjrt)."""
    import jax
    import numpy as _np
    from jax.sharding import Mesh, PartitionSpec, NamedSharding
    from jax.experimental.shard_map import shard_map
    from concourse import bass2jax, mybir as _mb
    bass2jax.install_neuronx_cc_hook()
    pname = nc.partition_id_tensor.name if nc.partition_id_tensor else None
    in_names, out_names, out_avals, zero_outs = [], [], [], []
    for alloc in nc.m.functions[0].allocations:
        if not isinstance(alloc, _mb.MemoryLocationSet):
            continue
        name = alloc.memorylocations[0].name
        if alloc.kind == "ExternalInput":
            if name != pname:
                in_names.append(name)
        elif alloc.kind == "ExternalOutput":
            out_names.append(name)
            shape = tuple(alloc.tensor_shape)
            dtype = _mb.dt.np(alloc.dtype)
            out_avals.append(jax.core.ShapedArray(shape, dtype))
            zero_outs.append(_np.zeros(shape, dtype))
    n_params = len(in_names)
    all_names = in_names + out_names
    if pname is not None:
        all_names = all_names + [pname]

    def _body(*args):
        outs = bass2jax._bass_exec_p.bind(
            *args, out_avals=tuple(out_avals), in_names=tuple(all_names),
            out_names=tuple(out_names), lowering_input_output_aliases=(),
            sim_require_finite=True, sim_require_nnan=True, nc=nc)
        return tuple(outs)

    n_extra = 1 if pname is not None else 0
    devices = jax.devices()[:n_cores]
    mesh = Mesh(_np.asarray(devices), ("core",))
    spec = NamedSharding(mesh, PartitionSpec("core"))
    sharded = jax.jit(
        shard_map(_body, mesh=mesh,
                  in_specs=(PartitionSpec("core"),) * (n_params + len(out_names) + n_extra),
                  out_specs=(PartitionSpec("core"),) * len(out_names),
                  check_rep=False),
        keep_unused=True)
    extra = ([np.concatenate([np.array([[c]], np.uint32) for c in range(n_cores)],
                              axis=0)] if pname is not None else [])
    return sharded, in_names, out_names, zero_outs, spec, extra


def benchmark(iters=5, **inputs):
    """Compile once, stage inputs on device, time executions. Returns
    (per-iter seconds list, outputs tuple like kernel())."""
    import time as _t
    import jax
    cfg = FULL
    D, H, W = cfg["D"], cfg["H"], cfg["W"]
    slab = D // 4
    inp = {k: np.asarray(v, dtype=np.float32) for k, v in inputs.items()}
    layers = prep_layers(inp)
    ins = make_core_inputs(cfg, inp["x"], inp["t"], layers,
                           inp["down_w"], inp["down_gamma"], inp["down_beta"])
    key = (D, H, W)
    if key not in _PROG_CACHE:
        _PROG_CACHE[key] = build_program(cfg)
    nc = _PROG_CACHE[key]
    sharded, in_names, out_names, zero_outs, spec, extra = _make_runner(nc, 8)
    concat_in = [np.concatenate([np.asarray(ins[c][n]) for c in range(8)], axis=0)
                 for n in in_names]
    concat_zero = [np.zeros((8 * z.shape[0], *z.shape[1:]), z.dtype)
                   for z in zero_outs]
    dev_in = [jax.device_put(a, spec) for a in concat_in]
    dev_zero = [jax.device_put(a, spec) for a in (concat_zero + extra)]
    jax.block_until_ready(dev_in)
    jax.block_until_ready(dev_zero)
    times = []
    outs = None
    for it in range(iters):
        t0 = _t.perf_counter()
        outs = sharded(*dev_in, *dev_zero)
        jax.block_until_ready(outs)
        times.append(_t.perf_counter() - t0)
    # assemble outputs like kernel()
    res = [np.asarray(o) for o in outs]
    by_name = {n: res[i] for i, n in enumerate(out_names)}
    N = 2
    xskip = np.zeros((N, CO, D, H, W), np.float32)
    y = np.zeros((N, CO, D // 2, H // 2, W // 2), np.float32)
    xs_all = by_name["xskip"].reshape(8, CO, slab, H, W)
    yo_all = by_name["yout"].reshape(8, CO, slab // 2, H // 2, W // 2)
    for core in range(8):
        n, s = divmod(core, 4)
        xskip[n][:, slab * s:slab * (s + 1)] = xs_all[core]
        y[n][:, (slab // 2) * s:(slab // 2) * (s + 1)] = yo_all[core]
    return times, (y, xskip)


# revision 16
# speedup vs baseline: 7884.5860x; 1.4681x over previous
"""Trainium2 Bass kernel for nn_MoDEEncoderBlock (MoDE encoder block).

Sharding: 8 cores = 2 samples x 4 D-slabs (16 output planes each).
Each core runs: mode_conv1 (with 2-plane halo recompute) -> instance_norm+mish
-> mode_conv2 -> instance_norm+mish (= x_skip) -> strided down conv ->
instance_norm+mish (= y). Instance-norm statistics are globalized with a tiny
AllReduce among each sample's 4 cores.

Conv = 125 shifted matmuls accumulating in PSUM, K-stacked (layer1: 4 kh-shifted
copies x 32ci = K128 quads; layer2: 2 copies x 64ci = K128 pairs), with two
512-position output blocks running concurrently via PE column tiling. Matmuls
use float32r (full-rate fp32).
"""
import os
import sys

for _p in ("/opt/trn_rl_repo", "/root/.axon_site/_ro/trn_rl_repo"):
    if os.path.isdir(_p) and _p not in sys.path:
        sys.path.insert(0, _p)

import numpy as np
import concourse.bacc as bacc
import concourse.mybir as mybir
import concourse.tile as tile
from concourse import bass_utils

f32 = mybir.dt.float32
f32r = mybir.dt.float32r
AF = mybir.ActivationFunctionType
ALU = mybir.AluOpType

EPS = 1e-5
CO = 64
CI1 = 32
T_DIM = 10
RG = [[0, 1, 2, 3], [4, 5, 6, 7]]

FULL = dict(D=64, H=64, W=64)
MINI = dict(D=16, H=16, W=16)


# ----------------------------------------------------------------------------
# host-side weight preparation (pure layout/static-weight transforms)
# ----------------------------------------------------------------------------

def _pad_k(k, p):
    return np.pad(k, ((0, 0), (0, 0), (p, p), (p, p), (p, p)))


def expert_stack(c5, c3, c1, a3, a5):
    """(5, Co, Ci, 5, 5, 5) expert kernel stack, mirrors reference."""
    pool3 = np.full((3, 3, 3), np.float32(1.0) / np.float32(27.0), np.float32)
    pool5 = np.full((5, 5, 5), np.float32(1.0) / np.float32(125.0), np.float32)
    return np.stack([
        c5,
        _pad_k(c3, 1),
        _pad_k(c1, 2),
        _pad_k(a3 * pool3, 1),
        a5 * pool5,
    ], axis=0).astype(np.float32)


def expert_lhsT_l1(E):
    """(5, 128, 34*64): kh0-3 quads (partition-stacked) + kh=4 singles
    packed 3-per-column on row-groups 0/32/64."""
    quad = E[:, :, :, :, 0:4, :]                       # e co ci kd j kw
    quad = quad.transpose(0, 4, 2, 3, 5, 1)            # e j ci kd kw co
    quad = quad.reshape(5, 4 * CI1, 25 * CO)
    sing = E[:, :, :, :, 4, :]                         # e co ci kd kw
    sing = sing.transpose(0, 2, 3, 4, 1).reshape(5, CI1, 25, CO)
    sing_f = np.zeros((5, 128, 9 * CO), np.float32)
    for i, (g, sl) in enumerate(((0, slice(0, 9)), (1, slice(9, 17)),
                                 (2, slice(17, 25)))):
        cnt = sl.stop - sl.start
        sing_f[:, 32 * g:32 * g + CI1, 0:cnt * CO] = \
            sing[:, :, sl].reshape(5, CI1, cnt * CO)
    return np.concatenate([quad, sing_f], axis=2).astype(np.float32)


def expert_lhsT_l2(E):
    """(5, 128, 63*64): kh-pairs (2-stacked) + kh=4 singles packed
    2-per-column on row-groups 0/64 (13 + 12)."""
    ci = CO
    pair = E[:, :, :, :, 0:4, :].reshape(5, CO, ci, 5, 2, 2, 5)  # e co ci kd half j kw
    pair = pair.transpose(0, 5, 2, 3, 6, 4, 1)                   # e j ci kd kw half co
    pair = pair.reshape(5, 2 * ci, 50 * CO)
    sing = E[:, :, :, :, 4, :].transpose(0, 2, 3, 4, 1).reshape(5, ci, 25, CO)
    sing_f = np.zeros((5, 128, 13 * CO), np.float32)
    sing_f[:, 0:ci, 0:13 * CO] = sing[:, :, 0:13].reshape(5, ci, 13 * CO)
    sing_f[:, ci:2 * ci, 0:12 * CO] = sing[:, :, 13:25].reshape(5, ci, 12 * CO)
    return np.concatenate([pair, sing_f], axis=2).astype(np.float32)


def down_lhsT(dw):
    """(128, 4*64): kd-pairs stacked; col (kh*2+kw)*64+co."""
    # dw: (Co, Ci=64, 2, 2, 2)
    a = dw.transpose(2, 1, 3, 4, 0)          # kd ci kh kw co
    return a.reshape(128, 4 * CO).astype(np.float32)


def gate_lhsT(gw, gb):
    """(11, 5*64): row k<10 = gw[:, k]; row 10 = gb."""
    out = np.zeros((T_DIM + 1, 5 * CO), np.float32)
    out[:T_DIM] = gw.T
    out[T_DIM] = gb
    return out


# ----------------------------------------------------------------------------
# device program
# ----------------------------------------------------------------------------

def build_program(cfg):
    D, H, W = cfg["D"], cfg["H"], cfg["W"]
    slab = D // 4
    HP, WP = H + 4, W + 4
    PS = HP * WP
    P1 = slab + 4            # conv1 output planes (incl 2+2 halo)
    PX = slab + 8            # x input planes
    nblk = H // 8            # 8-row blocks per plane
    npair = nblk // 2
    NB = 8 * W               # psum free size per block
    HD, WD = H // 2, W // 2
    dbr = min(HD, 512 // WD)     # down block rows
    nblkd = HD // dbr            # blocks per down plane (2 full, 1 mini)
    DP = slab // 2
    NBD = dbr * WD
    N1 = D * H * W
    ND = (D // 2) * (H // 2) * (W // 2)
    L1C = 34 * CO
    L2C = 63 * CO

    nc = bacc.Bacc("TRN2", target_bir_lowering=False, debug=False,
                   enable_asserts=False, num_devices=8)

    # ---- I/O ----
    xin = nc.dram_tensor("xin", [CI1, PX, HP, WP], f32, kind="ExternalInput").ap()
    taug = nc.dram_tensor("taug", [T_DIM + 1, 1], f32, kind="ExternalInput").ap()
    e1 = nc.dram_tensor("e1", [5, 128, L1C], f32, kind="ExternalInput").ap()
    e2 = nc.dram_tensor("e2", [5, 128, L2C], f32, kind="ExternalInput").ap()
    gw1 = nc.dram_tensor("gw1", [T_DIM + 1, 5 * CO], f32, kind="ExternalInput").ap()
    gw2 = nc.dram_tensor("gw2", [T_DIM + 1, 5 * CO], f32, kind="ExternalInput").ap()
    wdn = nc.dram_tensor("wdn", [128, 4 * CO], f32, kind="ExternalInput").ap()
    aff = nc.dram_tensor("aff", [CO, 6], f32, kind="ExternalInput").ap()
    mask = nc.dram_tensor("mask", [CO, 2], f32, kind="ExternalInput").ap()
    xskip = nc.dram_tensor("xskip", [CO, slab, H, W], f32, kind="ExternalOutput").ap()
    yout = nc.dram_tensor("yout", [CO, DP, HD, WD], f32, kind="ExternalOutput").ap()

    from contextlib import ExitStack
    with tile.TileContext(nc) as tc, ExitStack() as ctx:
        plane = ctx.enter_context(tc.tile_pool(name="plane", bufs=6))
        wrp = ctx.enter_context(tc.tile_pool(name="wrp", bufs=1))
        chp = ctx.enter_context(tc.tile_pool(name="chp", bufs=2))
        evp = ctx.enter_context(tc.tile_pool(name="evp", bufs=3))
        stp = ctx.enter_context(tc.tile_pool(name="stp", bufs=1))
        nop = ctx.enter_context(tc.tile_pool(name="nop", bufs=2))
        gp = ctx.enter_context(tc.tile_pool(name="gp", bufs=1))
        ps = ctx.enter_context(tc.tile_pool(name="ps", bufs=2, space="PSUM"))
        psg = ctx.enter_context(tc.tile_pool(name="psg", bufs=1, space="PSUM"))
        dr = ctx.enter_context(tc.tile_pool(name="dr", bufs=1, space="DRAM"))

        # ---- DRAM scratch ----
        y1_d = [dr.tile([CO, H * W], f32, tag=f"y1_{p}", name=f"y1_{p}") for p in range(P1)]
        hext = [dr.tile([CO, PS], f32, tag=f"hx_{p}", name=f"hx_{p}") for p in range(P1)]
        y2_d = [dr.tile([CO, H * W], f32, tag=f"y2_{p}", name=f"y2_{p}") for p in range(slab)]
        y3_d = [dr.tile([CO, nblkd * NBD], f32, tag=f"y3_{p}", name=f"y3_{p}") for p in range(DP)]
        w_d = [dr.tile([CO, H * W], f32, tag=f"wd_{p}", name=f"wd_{p}") for p in range(P1)]
        w3_d = [dr.tile([CO, nblkd * NBD], f32, tag=f"w3_{p}", name=f"w3_{p}") for p in range(DP)]
        g_d = [dr.tile([CO, 8], f32, tag=f"g_{l}", name=f"g_{l}") for l in range(2)]
        wsyn_d = [dr.tile([128, L1C], f32, tag="ws1", name="ws1"),
                  dr.tile([128, L2C], f32, tag="ws2", name="ws2")]
        ar_in = [dr.tile([CO, 2], f32, tag=f"ari{l}", name=f"ari{l}") for l in range(3)]
        ar_out = [dr.tile([CO, 2], f32, tag=f"aro{l}", name=f"aro{l}")
                  for l in range(3)]

        # ---- constants / small tiles ----
        afft = gp.tile([CO, 6], f32, tag="afft")
        nc.sync.dma_start(afft[:], aff)
        maskt = gp.tile([CO, 2], f32, tag="maskt")
        nc.sync.dma_start(maskt[:], mask)
        taut = gp.tile([T_DIM + 1, 1], f32, tag="taut")
        nc.sync.dma_start(taut[:], taug)
        ones1 = gp.tile([1, 128], f32, tag="ones1")
        nc.vector.memset(ones1[:], 1.0)

        # zero h_ext padding (borders). Use a zero tile, DMA over all hext planes.
        zt = nop.tile([CO, 2048], f32, tag="ni")
        nc.vector.memset(zt[:], 0.0)
        for p in range(P1):
            c0 = 0
            while c0 < PS:
                cw = min(2048, PS - c0)
                nc.sync.dma_start(hext[p][:, c0:c0 + cw], zt[:, 0:cw])
                c0 += cw

        # ---------------- gate + weight synthesis (per mode_conv layer) -------
        def gate_and_synth(l, gw_ap, e_ap, COLS, wr_tile):
            gwt = gp.tile([T_DIM + 1, 5 * CO], f32, tag="gwt")
            nc.sync.dma_start(gwt[:], gw_ap)
            zg = psg.tile([CO, 512], f32, tag="zg")
            for e in range(5):
                nc.tensor.matmul(zg[:, e:e + 1], gwt[0:T_DIM + 1, e * CO:(e + 1) * CO],
                                 taut[0:T_DIM + 1, 0:1],
                                 start=(e == 0), stop=(e == 4), skip_group_check=True)
            gexp = gp.tile([CO, 8], f32, tag="gexp")
            nc.scalar.activation(gexp[:, 0:5], zg[:, 0:5], AF.Exp)
            gs = gp.tile([CO, 1], f32, tag="gs")
            nc.vector.reduce_sum(gs[:], gexp[:, 0:5], axis=mybir.AxisListType.X)
            gr = gp.tile([CO, 1], f32, tag="gr")
            nc.vector.reciprocal(gr[:], gs[:])
            gv = gp.tile([CO, 8], f32, tag="gv")
            nc.vector.tensor_scalar_mul(gv[:, 0:5], gexp[:, 0:5], gr[:])
            nc.sync.dma_start(g_d[l][:, 0:5], gv[:, 0:5])
            gT2 = gp.tile([1, 5 * CO], f32, tag=f"gT{l}", name=f"gT{l}")
            nc.sync.dma_start(gT2[0:1, 0:5 * CO].rearrange("p (e c) -> p e c", c=CO),
                              g_d[l][:, 0:5].transpose([1, 0]))

            # broadcast g rows across all 128 partitions via ones outer-product
            gbt = []
            for e in range(5):
                pb = psg.tile([128, CO], f32, tag="pbb", name="pb")
                nc.tensor.matmul(pb[:], ones1[0:1, 0:128], gT2[0:1, e * CO:(e + 1) * CO],
                                 start=True, stop=True, skip_group_check=True)
                gbe = gp.tile([128, CO], f32, tag=f"gb{e}", name=f"gb{e}")
                nc.scalar.copy(gbe[:], pb[:])
                gbt.append(gbe)
            # mix experts chunk-wise, stage through DRAM, reload as f32r
            c0 = 0
            while c0 < COLS:
                cw = min(512, COLS - c0)
                acc = chp.tile([128, 512], f32, tag="acc")
                for e in range(5):
                    ech = chp.tile([128, 512], f32, tag="ech")
                    nc.sync.dma_start(ech[:, 0:cw], e_ap[e, :, c0:c0 + cw])
                    gb = gbt[e][:, 0:CO].unsqueeze(1).broadcast_to([128, cw // CO, CO])
                    ev = ech[:, 0:cw].rearrange("p (q c) -> p q c", c=CO)
                    av = acc[:, 0:cw].rearrange("p (q c) -> p q c", c=CO)
                    if e == 0:
                        nc.vector.tensor_tensor(av, ev, gb, op=ALU.mult)
                    else:
                        tmp = chp.tile([128, 512], f32, tag="tmp")
                        tv = tmp[:, 0:cw].rearrange("p (q c) -> p q c", c=CO)
                        nc.vector.tensor_tensor(tv, ev, gb, op=ALU.mult)
                        nc.vector.tensor_tensor(av, av, tv, op=ALU.add)
                nc.sync.dma_start(wsyn_d[l][:, c0:c0 + cw], acc[:, 0:cw])
                c0 += cw
            nc.sync.dma_start(wr_tile[:], wsyn_d[l][:, :].bitcast(f32r))

        w1r = wrp.tile([128, L1C], f32r, tag="w1r")
        w2r = wrp.tile([128, L2C], f32r, tag="w2r")
        wdr = wrp.tile([128, 4 * CO], f32r, tag="wdr")
        nc.sync.dma_start(wdr[:], wdn.bitcast(f32r))
        gate_and_synth(0, gw1, e1, L1C, w1r)
        gate_and_synth(1, gw2, e2, L2C, w2r)

        # ---------------- stats helpers ----------------------------------
        def stats_to_scales(lidx, st, nA, n_total, g_col, b_col):
            """Reduce bn_stats tile [64, nA*6] -> AllReduce -> scale/bias."""
            tmp = stp.tile([CO, 2 * max(nA, 1)], f32, tag="stmp")
            v = st[:, 0:nA * 6].rearrange("p (n s) -> p n s", s=6)
            # s1 = sum(cnt*mean) over even+odd halves
            nc.vector.tensor_tensor(tmp[:, 0:nA], v[:, :, 0], v[:, :, 1], op=ALU.mult)
            nc.vector.tensor_tensor(tmp[:, nA:2 * nA], v[:, :, 3], v[:, :, 4], op=ALU.mult)
            sums = stp.tile([CO, 2], f32, tag="sums")
            nc.vector.reduce_sum(sums[:, 0:1], tmp[:, 0:2 * nA], axis=mybir.AxisListType.X)
            # s2 = sum(cnt*mean^2 + cnt*var)
            nc.vector.tensor_tensor(tmp[:, 0:nA], v[:, :, 1], v[:, :, 1], op=ALU.mult)
            nc.vector.tensor_tensor(tmp[:, 0:nA], tmp[:, 0:nA], v[:, :, 0], op=ALU.mult)
            nc.vector.tensor_tensor(tmp[:, 0:nA], tmp[:, 0:nA], v[:, :, 2], op=ALU.add)
            nc.vector.tensor_tensor(tmp[:, nA:2 * nA], v[:, :, 4], v[:, :, 4], op=ALU.mult)
            nc.vector.tensor_tensor(tmp[:, nA:2 * nA], tmp[:, nA:2 * nA], v[:, :, 3], op=ALU.mult)
            nc.vector.tensor_tensor(tmp[:, nA:2 * nA], tmp[:, nA:2 * nA], v[:, :, 5], op=ALU.add)
            nc.vector.reduce_sum(sums[:, 1:2], tmp[:, 0:2 * nA], axis=mybir.AxisListType.X)
            nc.sync.dma_start(ar_in[lidx][:, :], sums[:])
            nc.gpsimd.collective_compute(
                "AllReduce", ALU.add, replica_groups=RG,
                ins=[ar_in[lidx][:, :]], outs=[ar_out[lidx][:, :]])
            tot = stp.tile([CO, 2], f32, tag="tot")
            nc.sync.dma_start(tot[:], ar_out[lidx][:, :])
            mu = stp.tile([CO, 1], f32, tag=f"mu{lidx}")
            nc.vector.tensor_scalar_mul(mu[:], tot[:, 0:1], 1.0 / n_total)
            e2t = stp.tile([CO, 1], f32, tag="e2t")
            nc.vector.tensor_scalar_mul(e2t[:], tot[:, 1:2], 1.0 / n_total)
            var = stp.tile([CO, 1], f32, tag="var")
            nc.vector.tensor_tensor(var[:], mu[:], mu[:], op=ALU.mult)
            nc.vector.tensor_tensor(var[:], e2t[:], var[:], op=ALU.subtract)
            nc.vector.tensor_scalar_add(var[:], var[:], EPS)
            sd = stp.tile([CO, 1], f32, tag="sd")
            nc.scalar.sqrt(sd[:], var[:])
            inv = stp.tile([CO, 1], f32, tag="inv")
            nc.vector.reciprocal(inv[:], sd[:])
            s_ch = stp.tile([CO, 1], f32, tag=f"s{lidx}")
            nc.vector.tensor_tensor(s_ch[:], inv[:], afft[:, g_col:g_col + 1], op=ALU.mult)
            negs = stp.tile([CO, 1], f32, tag="negs")
            nc.vector.tensor_scalar_mul(negs[:], s_ch[:], -1.0)
            b_ch = stp.tile([CO, 1], f32, tag=f"b{lidx}")
            nc.vector.scalar_tensor_tensor(b_ch[:], mu[:], negs[:],
                                           afft[:, b_col:b_col + 1],
                                           op0=ALU.mult, op1=ALU.add)
            return s_ch, b_ch

        # ---------------- conv layers ----------------------------------
        def conv_layer(l, wr, n_out, src_plane_ap, groups, y_planes,
                       st, owned_range):
            """l=0: Ci=32, kh-quads K=128 + 3-way row-tiled kh4 singles.
            l=1: Ci=64, kh-pairs K=128 + 2-way row-tiled kh4 singles."""
            ci = CI1 if l == 0 else CO
            nstk = 25 if l == 0 else 50
            splits = (9, 8, 8) if l == 0 else (13, 12)
            # singles assignment: (group, col-slot, kd, kw); bank 0 shares pP
            assign = []
            t = 0
            for g, cnt in enumerate(splits):
                for sg in range(cnt):
                    kd, kw = divmod(t, 5)
                    assign.append((g, sg, kd, kw))
                    t += 1
            # interleave groups for PE row-tile concurrency
            inter = []
            for sslot in range(max(splits)):
                for g in range(len(splits)):
                    if sslot < splits[g]:
                        inter.append(assign[sum(splits[:g]) + sslot])
            lastg = {}
            for i, (g, sg, kd, kw) in enumerate(inter):
                lastg[g] = i
            tiles = {}

            def ensure(p):
                if p in tiles:
                    return
                tpl = plane.tile([128, PS], f32r, tag="plane", name="xpl")
                for g in range(groups):
                    nc.sync.dma_start(
                        tpl[ci * g:ci * g + ci, 0:PS - g * WP],
                        src_plane_ap(p)[:, g * WP:PS].bitcast(f32r))
                tiles[p] = tpl

            iA = 0
            n_src = n_out + 4
            for dp in range(n_out):
                want = range(dp, min(dp + 6, n_src)) if dp == 0 else \
                    [p for p in (dp + 5,) if p < n_src]
                for p in want:
                    ensure(p)
                for blk in range(nblk):
                    h0 = blk * 8
                    pP = ps.tile([128, NB], f32, tag="psA", name="pP")
                    pQ = ps.tile([128, NB], f32, tag="psB", name="pQ")
                    pR = (ps.tile([128, NB], f32, tag="psC", name="pR")
                          if l == 0 else None)
                    bankof = (pP, pQ, pR) if l == 0 else (pP, pQ)
                    # K=128 tap-stacked matmuls -> pP
                    for q in range(nstk):
                        if l == 0:
                            kd, kw = divmod(q, 5)
                            ro = h0
                        else:
                            kk, half = divmod(q, 2)
                            kd, kw = divmod(kk, 5)
                            ro = h0 + 2 * half
                        lhsT = wr[:, q * CO:(q + 1) * CO]
                        xv = tiles[dp + kd][:, :].rearrange("p (h w) -> p h w", w=WP)
                        nc.tensor.matmul(pP[0:64, :], lhsT,
                                         xv[:, ro:ro + 8, kw:kw + W],
                                         start=(q == 0), stop=False,
                                         tile_position=(0, 0), skip_group_check=True)
                    # kh=4 singles, row-tiled into per-group banks
                    for i, (g, sg, kd, kw) in enumerate(inter):
                        col = (nstk + sg) * CO
                        lhsT = wr[ci * g:ci * (g + 1), col:col + CO]
                        bank = bankof[g]
                        base_r = h0 + 4 - g
                        xv = tiles[dp + kd][ci * g:ci * (g + 1), :].rearrange(
                            "p (h w) -> p h w", w=WP)
                        nc.tensor.matmul(bank[0:64, :], lhsT,
                                         xv[:, base_r:base_r + 8, kw:kw + W],
                                         start=(g > 0 and sg == 0),
                                         stop=(i == lastg[g]),
                                         tile_position=(ci * g, 0),
                                         skip_group_check=True)
                    # merge banks + evacuate (one PSUM operand per DVE op)
                    ev = evp.tile([CO, NB], f32, tag="ev")
                    nc.scalar.copy(ev[:], pP[0:64, :])
                    nc.vector.tensor_tensor(ev[:], ev[:], pQ[0:64, :], op=ALU.add)
                    if l == 0:
                        nc.vector.tensor_tensor(ev[:], ev[:], pR[0:64, :],
                                                op=ALU.add)
                    nc.sync.dma_start(y_planes[dp][:, h0 * W:h0 * W + NB], ev[:])
                    if owned_range[0] <= dp < owned_range[1]:
                        nc.vector.bn_stats(st[:, iA * 6:(iA + 1) * 6], ev[:])
                        iA += 1
            return iA

        # ---------------- normalize + mish sweeps ----------------------
        def norm_mish(y_planes, out_fn, s_list, b_list, batch, half_cols, parts):
            """Two-sweep mish over plane list; s_list/b_list per plane."""
            n = len(y_planes)
            halves = [(p, c0) for p in range(n)
                      for c0 in range(0, y_planes[p].shape[1], half_cols)]
            for b0 in range(0, len(halves), batch):
                grp = halves[b0:b0 + batch]
                for (p, c0) in grp:   # sweep A: u=exp(y*s+b); w=ln(u+1)
                    yt = nop.tile([parts, half_cols], f32, tag="ni")
                    nc.sync.dma_start(yt[:], y_planes[p][0:parts, c0:c0 + half_cols])
                    ut = nop.tile([parts, half_cols], f32, tag="nu")
                    nc.scalar.activation(ut[:], yt[:], AF.Exp,
                                         bias=b_list[p][0:parts, :], scale=s_list[p][0:parts, :])
                    nc.vector.tensor_scalar_add(ut[:], ut[:], 1.0)
                    nc.scalar.activation(ut[:], ut[:], AF.Ln)
                    wdst = w_d[p] if parts == CO else w3_d[p]
                    nc.sync.dma_start(wdst[:, c0:c0 + half_cols][0:parts, :], ut[:])
                for (p, c0) in grp:   # sweep B: h = (y*s+b) * tanh(w)
                    wt = nop.tile([parts, half_cols], f32, tag="nu")
                    wsrc = w_d[p] if parts == CO else w3_d[p]
                    nc.sync.dma_start(wt[:], wsrc[:, c0:c0 + half_cols][0:parts, :])
                    nc.scalar.activation(wt[:], wt[:], AF.Tanh)
                    yt = nop.tile([parts, half_cols], f32, tag="ni")
                    nc.sync.dma_start(yt[:], y_planes[p][0:parts, c0:c0 + half_cols])
                    nc.vector.tensor_scalar(yt[:], yt[:], s_list[p][0:parts, :],
                                            b_list[p][0:parts, :],
                                            op0=ALU.mult, op1=ALU.add)
                    nc.gpsimd.tensor_tensor(yt[:], yt[:], wt[:], op=ALU.mult)
                    out_fn(p, c0, yt)

        # ================= layer 1 =================
        st1 = stp.tile([CO, max(slab * nblk, 1) * 6], f32, tag="st1")
        nc.vector.memset(st1[:], 0.0)
        nA1 = conv_layer(0, w1r, P1, lambda p: xin[:, p].rearrange("c h w -> c (h w)"),
                         4, y1_d, st1, (2, 2 + slab))
        s1, b1 = stats_to_scales(0, st1, nA1, N1, 0, 1)
        # halo-plane masked scale/bias (edge cores zero their out-of-range planes)
        mlo = maskt[:, 0:1]
        mhi = maskt[:, 1:2]
        s1lo = stp.tile([CO, 1], f32, tag="s1lo")
        b1lo = stp.tile([CO, 1], f32, tag="b1lo")
        s1hi = stp.tile([CO, 1], f32, tag="s1hi")
        b1hi = stp.tile([CO, 1], f32, tag="b1hi")
        nc.vector.tensor_tensor(s1lo[:], s1[:], mlo, op=ALU.mult)
        nc.vector.tensor_tensor(b1lo[:], b1[:], mlo, op=ALU.mult)
        nc.vector.tensor_tensor(s1hi[:], s1[:], mhi, op=ALU.mult)
        nc.vector.tensor_tensor(b1hi[:], b1[:], mhi, op=ALU.mult)
        s_l1 = [s1lo, s1lo] + [s1] * (P1 - 4) + [s1hi, s1hi]
        b_l1 = [b1lo, b1lo] + [b1] * (P1 - 4) + [b1hi, b1hi]

        def h_out(p, c0, yt):
            # write into hext interior: rows c0//W .. +half rows, cols 2..2+W
            r0 = c0 // W
            nrow = yt.shape[1] // W
            dst = hext[p][:, :].rearrange("c (h w) -> c h w", w=WP)
            dst = dst[:, 2 + r0:2 + r0 + nrow, 2:2 + W]
            nc.sync.dma_start(dst, yt[:].rearrange("c (h w) -> c h w", w=W))

        norm_mish(y1_d, h_out, s_l1, b_l1, batch=8, half_cols=H * W // 2, parts=CO)

        # ================= layer 2 =================
        st2 = stp.tile([CO, max(slab * nblk, 1) * 6], f32, tag="st2")
        nc.vector.memset(st2[:], 0.0)
        nA2 = conv_layer(1, w2r, slab, lambda p: hext[p][:, :],
                         2, y2_d, st2, (0, slab))
        s2, b2 = stats_to_scales(1, st2, nA2, N1, 2, 3)

        def xs_out(p, c0, yt):
            nc.sync.dma_start(xskip[:, p].rearrange("c h w -> c (h w)")[:, c0:c0 + yt.shape[1]], yt[:])

        norm_mish(y2_d, xs_out, [s2] * slab, [b2] * slab,
                  batch=8, half_cols=H * W // 2, parts=CO)

        # ================= down layer =================
        std = stp.tile([CO, max(DP * nblkd, 1) * 6], f32, tag="std")
        nc.vector.memset(std[:], 0.0)
        iD = 0
        for dd in range(DP):
            xdt = plane.tile([128, H * W], f32r, tag="plane", name="xdt")
            nc.sync.dma_start(xdt[0:64, :],
                              xskip[:, 2 * dd].rearrange("c h w -> c (h w)").bitcast(f32r))
            nc.sync.dma_start(xdt[64:128, :],
                              xskip[:, 2 * dd + 1].rearrange("c h w -> c (h w)").bitcast(f32r))
            xv = xdt[:, :].rearrange("p (h th w tw) -> p h th w tw", th=2, tw=2, w=WD)
            for blk in range(nblkd):
                r0 = blk * dbr
                pP = ps.tile([128, NBD], f32, tag="psA", name="pPd")
                for g in range(4):
                    kh, kw = divmod(g, 2)
                    lhsT = wdr[:, g * CO:(g + 1) * CO]
                    nc.tensor.matmul(pP[0:64, :], lhsT,
                                     xv[:, r0:r0 + dbr, kh, :, kw],
                                     start=(g == 0), stop=(g == 3),
                                     tile_position=(0, 0), skip_group_check=True)
                ev = evp.tile([CO, NBD], f32, tag="ev")
                nc.scalar.copy(ev[:], pP[0:64, :])
                nc.sync.dma_start(y3_d[dd][:, blk * NBD:(blk + 1) * NBD], ev[:])
                nc.vector.bn_stats(std[:, iD * 6:(iD + 1) * 6], ev[:])
                iD += 1
        sdn, bdn = stats_to_scales(2, std, iD, ND, 4, 5)

        def y_out(p, c0, yt):
            blk = c0 // NBD
            yv = yt[:].rearrange("c (h w) -> c h w", w=WD)
            nc.sync.dma_start(yout[:, p, blk * dbr:(blk + 1) * dbr, :], yv)

        norm_mish(y3_d, y_out, [sdn] * DP, [bdn] * DP,
                  batch=8, half_cols=NBD, parts=CO)

    nc.compile()
    return nc


# ----------------------------------------------------------------------------
# host entry
# ----------------------------------------------------------------------------

def make_core_inputs(cfg, x, t, layers, down_w, down_gamma, down_beta):
    """Per-core input dicts (8 cores = 2 samples x 4 slabs)."""
    D, H, W = cfg["D"], cfg["H"], cfg["W"]
    slab = D // 4
    (E1, gwt1, g1, b1), (E2, gwt2, g2, b2) = layers
    e1 = expert_lhsT_l1(E1)
    e2 = expert_lhsT_l2(E2)
    wd = down_lhsT(down_w)
    aff = np.stack([g1, b1, g2, b2, down_gamma, down_beta], axis=1).astype(np.float32)
    ins = []
    for core in range(8):
        n, s = divmod(core, 4)
        xp = np.pad(x[n], ((0, 0), (4, 4), (2, 2), (2, 2)))
        xin = np.ascontiguousarray(xp[:, slab * s: slab * s + slab + 8])
        ta = np.concatenate([t[n], [1.0]]).astype(np.float32).reshape(T_DIM + 1, 1)
        m = np.repeat(np.array([[0.0 if s == 0 else 1.0,
                                 0.0 if s == 3 else 1.0]], np.float32), 64, axis=0)
        ins.append({
            "xin": xin.astype(np.float32), "taug": ta,
            "e1": e1, "e2": e2, "gw1": gwt1, "gw2": gwt2,
            "wdn": wd, "aff": aff, "mask": m,
        })
    return ins


def prep_layers(inp):
    E1 = expert_stack(inp["l1_conv5"], inp["l1_conv3"], inp["l1_conv1"],
                      inp["l1_avg3"], inp["l1_avg5"])
    E2 = expert_stack(inp["l2_conv5"], inp["l2_conv3"], inp["l2_conv1"],
                      inp["l2_avg3"], inp["l2_avg5"])
    gw1 = gate_lhsT(inp["l1_gw"], inp["l1_gb"])
    gw2 = gate_lhsT(inp["l2_gw"], inp["l2_gb"])
    return ((E1, gw1, inp["l1_gamma"], inp["l1_beta"]),
            (E2, gw2, inp["l2_gamma"], inp["l2_beta"]))


_PROG_CACHE = {}
LAST_EXEC_NS = None


def kernel(**inputs):
    global LAST_EXEC_NS
    cfg = FULL
    D, H, W = cfg["D"], cfg["H"], cfg["W"]
    slab = D // 4
    inp = {k: np.asarray(v, dtype=np.float32) for k, v in inputs.items()}
    layers = prep_layers(inp)
    ins = make_core_inputs(cfg, inp["x"], inp["t"], layers,
                           inp["down_w"], inp["down_gamma"], inp["down_beta"])
    key = (D, H, W)
    if key not in _PROG_CACHE:
        _PROG_CACHE[key] = build_program(cfg)
    nc = _PROG_CACHE[key]
    trace = os.environ.get("BASS_KERNEL_PROFILE", "0") == "1"
    try:
        res = bass_utils.run_bass_kernel_spmd(nc, ins, core_ids=list(range(8)),
                                              trace=trace)
    except ModuleNotFoundError:
        res = bass_utils.run_bass_kernel_spmd(nc, ins, core_ids=list(range(8)),
                                              trace=False)
    LAST_EXEC_NS = res.exec_time_ns
    N = 2
    xskip = np.zeros((N, CO, D, H, W), np.float32)
    y = np.zeros((N, CO, D // 2, H // 2, W // 2), np.float32)
    for core in range(8):
        n, s = divmod(core, 4)
        r = res.results[core]
        xskip[n][:, slab * s:slab * (s + 1)] = r["xskip"].reshape(CO, slab, H, W)
        y[n][:, (slab // 2) * s:(slab // 2) * (s + 1)] = \
            r["yout"].reshape(CO, slab // 2, H // 2, W // 2)
    return (y, xskip)


def _make_runner(nc, n_cores):
    """Persistent compiled runner (mirrors bass2jax.run_bass_via_pjrt)."""
    import jax
    import numpy as _np
    from jax.sharding import Mesh, PartitionSpec, NamedSharding
    from jax.experimental.shard_map import shard_map
    from concourse import bass2jax, mybir as _mb
    bass2jax.install_neuronx_cc_hook()
    pname = nc.partition_id_tensor.name if nc.partition_id_tensor else None
    in_names, out_names, out_avals, zero_outs = [], [], [], []
    for alloc in nc.m.functions[0].allocations:
        if not isinstance(alloc, _mb.MemoryLocationSet):
            continue
        name = alloc.memorylocations[0].name
        if alloc.kind == "ExternalInput":
            if name != pname:
                in_names.append(name)
        elif alloc.kind == "ExternalOutput":
            out_names.append(name)
            shape = tuple(alloc.tensor_shape)
            dtype = _mb.dt.np(alloc.dtype)
            out_avals.append(jax.core.ShapedArray(shape, dtype))
            zero_outs.append(_np.zeros(shape, dtype))
    n_params = len(in_names)
    all_names = in_names + out_names
    if pname is not None:
        all_names = all_names + [pname]

    def _body(*args):
        outs = bass2jax._bass_exec_p.bind(
            *args, out_avals=tuple(out_avals), in_names=tuple(all_names),
            out_names=tuple(out_names), lowering_input_output_aliases=(),
            sim_require_finite=True, sim_require_nnan=True, nc=nc)
        return tuple(outs)

    n_extra = 1 if pname is not None else 0
    devices = jax.devices()[:n_cores]
    mesh = Mesh(_np.asarray(devices), ("core",))
    spec = NamedSharding(mesh, PartitionSpec("core"))
    sharded = jax.jit(
        shard_map(_body, mesh=mesh,
                  in_specs=(PartitionSpec("core"),) * (n_params + len(out_names) + n_extra),
                  out_specs=(PartitionSpec("core"),) * len(out_names),
                  check_rep=False),
        keep_unused=True)
    extra = ([np.concatenate([np.array([[c]], np.uint32) for c in range(n_cores)],
                              axis=0)] if pname is not None else [])
    return sharded, in_names, out_names, zero_outs, spec, extra


def benchmark(iters=5, **inputs):
    """Compile once, stage inputs on device, time executions. Returns
    (per-iter seconds list, outputs tuple like kernel())."""
    import time as _t
    import jax
    cfg = FULL
    D, H, W = cfg["D"], cfg["H"], cfg["W"]
    slab = D // 4
    inp = {k: np.asarray(v, dtype=np.float32) for k, v in inputs.items()}
    layers = prep_layers(inp)
    ins = make_core_inputs(cfg, inp["x"], inp["t"], layers,
                           inp["down_w"], inp["down_gamma"], inp["down_beta"])
    key = (D, H, W)
    if key not in _PROG_CACHE:
        _PROG_CACHE[key] = build_program(cfg)
    nc = _PROG_CACHE[key]
    sharded, in_names, out_names, zero_outs, spec, extra = _make_runner(nc, 8)
    concat_in = [np.concatenate([np.asarray(ins[c][n]) for c in range(8)], axis=0)
                 for n in in_names]
    concat_zero = [np.zeros((8 * z.shape[0], *z.shape[1:]), z.dtype)
                   for z in zero_outs]
    dev_in = [jax.device_put(a, spec) for a in concat_in]
    dev_zero = [jax.device_put(a, spec) for a in (concat_zero + extra)]
    jax.block_until_ready(dev_in)
    jax.block_until_ready(dev_zero)
    times = []
    outs = None
    for it in range(iters):
        t0 = _t.perf_counter()
        outs = sharded(*dev_in, *dev_zero)
        jax.block_until_ready(outs)
        times.append(_t.perf_counter() - t0)
    # assemble outputs like kernel()
    res = [np.asarray(o) for o in outs]
    by_name = {n: res[i] for i, n in enumerate(out_names)}
    N = 2
    xskip = np.zeros((N, CO, D, H, W), np.float32)
    y = np.zeros((N, CO, D // 2, H // 2, W // 2), np.float32)
    xs_all = by_name["xskip"].reshape(8, CO, slab, H, W)
    yo_all = by_name["yout"].reshape(8, CO, slab // 2, H // 2, W // 2)
    for core in range(8):
        n, s = divmod(core, 4)
        xskip[n][:, slab * s:slab * (s + 1)] = xs_all[core]
        y[n][:, (slab // 2) * s:(slab // 2) * (s + 1)] = yo_all[core]
    return times, (y, xskip)
